# revision 31
# baseline (speedup 1.0000x reference)
"""Two-layer GCN (PyG GCNConv x2 + ReLU) on 8 Trainium2 NeuronCores via Bass.

Formulation: GCN aggregation is linear row-mixing, so for each layer
    conv(H) = A_hat @ H @ W + b      (A_hat includes self-loops, sym-norm)
and we aggregate FIRST, then matmul:
    z   = relu(A_hat @ x @ W1 + b1)
    out = A_hat @ z @ W2 + b2

Sharding: nodes split into 8 row-slabs (2560 padded rows each). Each core
aggregates + matmuls its own dst rows. The only communication is one
AllGather of z (bf16) so every core can gather arbitrary src rows for L2.

Aggregation on device: edges are bucketed by 128-dst-node "supertiles",
padded to B 128-edge blocks per supertile. For each block, the gathered
src-rows tile G [128 edges, C] and a host-built selection matrix
S^T [128 edges, 128 dst] (entry [e, dstl] = norm(e)) produce
    agg^T[ch_tile, dst] += G[:, ch_tile].T @ S^T     (PSUM accumulate)
which directly yields agg^T as matmul lhsT for the subsequent @W.

L1 gathers are free: x is a kernel input, so the edge-ordered gather table
xe is prebuilt on host and streamed sequentially. L2 gathers rows of z_full
via indirect DMA (the only runtime gather).
"""

import numpy as np
import ml_dtypes

N_NODES = 20000
IN_CH = 256
HID_CH = 512
N_CORES = 8
P = 128
NPAD = 20480            # 160 supertiles of 128
NSUP = NPAD // P        # 160
SUP_PER_CORE = NSUP // N_CORES  # 20
ROWS_PER_CORE = NPAD // N_CORES  # 2560

bf16 = ml_dtypes.bfloat16

TRACE = False           # set by test harness for HW profiling
DEBUG_Z = False         # add a z_full dump output (debug only)
LAST_RESULTS = None     # BassKernelResults of the last device run

_COMPILED = {}          # B -> (nc, input names metadata)


def _preprocess(x, edge_index):
    """Build per-core gather/selection tables. Returns dict of host arrays."""
    src = edge_index[0].astype(np.int64)
    dst = edge_index[1].astype(np.int64)
    n = N_NODES

    deg = np.bincount(dst, minlength=n).astype(np.float32) + 1.0
    dinv = 1.0 / np.sqrt(deg)

    # self-loops (w = dinv^2) are handled as a per-partition DVE term, not
    # edges — keeps one gather descriptor per real edge only.
    allw = (dinv[src] * dinv[dst]).astype(np.float32)

    order = np.argsort(dst, kind="stable")
    s_src = src[order]
    s_dst = dst[order]
    s_w = allw[order]

    sup = (s_dst >> 7).astype(np.int64)          # supertile per edge
    cnt = np.bincount(sup, minlength=NSUP)
    B = int(np.ceil(cnt.max() / P))              # uniform blocks per supertile
    CAP = B * P

    starts = np.zeros(NSUP, np.int64)
    starts[1:] = np.cumsum(cnt)[:-1]
    slot = np.arange(len(s_dst)) - starts[sup]   # slot within supertile
    gslot = sup * CAP + slot                     # global padded slot

    esrc = np.zeros(NSUP * CAP, np.int32)        # padded: src=0, w=0
    esrc[gslot] = s_src.astype(np.int32)
    edstl = np.zeros(NSUP * CAP, np.int64)
    edstl[gslot] = s_dst & 127
    ew = np.zeros(NSUP * CAP, np.float32)
    ew[gslot] = s_w

    # S^T blocks: [NSUP, slot, dstl] -> DMA layout [NSUP, p, b*128+dstl]
    st = np.zeros((NSUP * CAP, P), bf16)
    st[np.arange(NSUP * CAP), edstl] = ew.astype(bf16)
    st = (
        st.reshape(NSUP, B, P, P)
        .transpose(0, 2, 1, 3)                   # [sup, p, b, dstl]
        .reshape(NSUP, P, B * P)
    )

    # L2 gather indices: [NSUP, p, b]
    zidx = esrc.reshape(NSUP, B, P).transpose(0, 2, 1).copy()

    # L1 pregathered edge-ordered x: [NSUP, p, b*IN_CH]
    xbf = np.ascontiguousarray(x.astype(bf16))
    xe = (
        xbf[esrc.reshape(NSUP, B, P)]            # [sup, b, p, IN_CH]
        .transpose(0, 2, 1, 3)
        .reshape(NSUP, P, B * IN_CH)
    )

    # self-loop term inputs: own x rows (node order) + dinv^2 per node
    xown = np.zeros((NPAD, IN_CH), bf16)
    xown[:n] = xbf
    xown = xown.reshape(NSUP, P, IN_CH)
    dinv2 = np.zeros((NPAD,), np.float32)
    dinv2[:n] = dinv * dinv
    dinv2 = dinv2.reshape(NSUP, P, 1)

    return {"B": B, "st": st, "zidx": zidx, "xe": xe, "xown": xown,
            "dinv2": dinv2}


def _build_program(B, has_bias):
    import concourse.bass as bass
    import concourse.mybir as mybir
    import concourse.tile as tile
    from concourse.bacc import Bacc
    from concourse.masks import make_identity

    dt = mybir.dt
    nc = Bacc("TRN2", target_bir_lowering=False, debug=False, num_devices=N_CORES)

    t_xe = nc.dram_tensor("xe", [SUP_PER_CORE, P, B * IN_CH], dt.bfloat16,
                          kind="ExternalInput")
    t_st = nc.dram_tensor("st", [SUP_PER_CORE, P, B * P], dt.bfloat16,
                          kind="ExternalInput")
    t_zidx = nc.dram_tensor("zidx", [SUP_PER_CORE, P, B], dt.int32,
                            kind="ExternalInput")
    t_xown = nc.dram_tensor("xown", [SUP_PER_CORE, P, IN_CH], dt.bfloat16,
                            kind="ExternalInput")
    t_dinv2 = nc.dram_tensor("dinv2", [SUP_PER_CORE, P, 1], dt.float32,
                             kind="ExternalInput")
    t_w1 = nc.dram_tensor("w1", [2, P, HID_CH], dt.bfloat16, kind="ExternalInput")
    t_w2 = nc.dram_tensor("w2", [4, P, HID_CH], dt.bfloat16, kind="ExternalInput")
    if has_bias:
        t_b1 = nc.dram_tensor("b1b", [P, HID_CH], dt.float32, kind="ExternalInput")
        t_b2 = nc.dram_tensor("b2b", [P, HID_CH], dt.float32, kind="ExternalInput")
    t_out = nc.dram_tensor("out", [ROWS_PER_CORE, HID_CH], dt.float32,
                           kind="ExternalOutput")
    t_zdbg = None
    if DEBUG_Z:
        t_zdbg = nc.dram_tensor("zdbg", [NPAD, HID_CH], dt.bfloat16,
                                kind="ExternalOutput")

    K1 = IN_CH // P   # 2 ch tiles in L1
    K2 = HID_CH // P  # 4 ch tiles in L2

    with tile.TileContext(nc) as tc:
        with (
            tc.tile_pool(name="dram", bufs=1, space="DRAM") as dram,
            tc.tile_pool(name="const", bufs=1) as cpool,
            tc.tile_pool(name="work", bufs=4) as pool,
            tc.tile_pool(name="big", bufs=3) as bigpool,
        ):
            z_slice = dram.tile([ROWS_PER_CORE, HID_CH], dt.bfloat16, name="z_slice")
            z_full = dram.tile([NPAD, HID_CH], dt.bfloat16, name="z_full",
                               addr_space="Shared")

            w1_t = cpool.tile([P, K1 * HID_CH], dt.bfloat16, name="w1_t")
            for m in range(K1):
                nc.sync.dma_start(out=w1_t[:, m * HID_CH:(m + 1) * HID_CH],
                                  in_=t_w1[m])
            w2_t = cpool.tile([P, K2 * HID_CH], dt.bfloat16, name="w2_t")
            for m in range(K2):
                nc.sync.dma_start(out=w2_t[:, m * HID_CH:(m + 1) * HID_CH],
                                  in_=t_w2[m])
            if has_bias:
                b1_t = cpool.tile([P, HID_CH], dt.float32, name="b1_t")
                nc.sync.dma_start(out=b1_t[:], in_=t_b1[:])
                b2_t = cpool.tile([P, HID_CH], dt.float32, name="b2_t")
                nc.sync.dma_start(out=b2_t[:], in_=t_b2[:])
            ident = cpool.tile([P, P], dt.float32, name="ident")
            make_identity(nc, ident[:])

            # ---------------- Layer 1 ----------------
            with tc.tile_pool(name="psum1", bufs=2, space="PSUM") as psum1:
                for s in range(SUP_PER_CORE):
                    xe_t = bigpool.tile([P, B * IN_CH], dt.bfloat16, tag="xe",
                                        name=f"xe{s}")
                    nc.sync.dma_start(out=xe_t[:], in_=t_xe[s])
                    st_t = bigpool.tile([P, B * P], dt.bfloat16, tag="st1",
                                        name=f"st{s}")
                    nc.sync.dma_start(out=st_t[:], in_=t_st[s])

                    # row-major aggregation: S^T stationary, gathered rows stream
                    ag1 = psum1.tile([P, IN_CH], dt.float32, tag="ag1",
                                     name=f"ag1_{s}")
                    for b in range(B):
                        nc.tensor.matmul(
                            out=ag1[:],
                            lhsT=st_t[:, b * P:(b + 1) * P],
                            rhs=xe_t[:, b * IN_CH:(b + 1) * IN_CH],
                            start=(b == 0),
                            stop=(b == B - 1),
                        )
                    xown_t = pool.tile([P, IN_CH], dt.bfloat16, tag="xown",
                                       name=f"xown{s}")
                    nc.sync.dma_start(out=xown_t[:], in_=t_xown[s])
                    dinv2_t = pool.tile([P, 1], dt.float32, tag="dinv2",
                                        name=f"dinv2{s}")
                    nc.sync.dma_start(out=dinv2_t[:], in_=t_dinv2[s])
                    self1 = pool.tile([P, IN_CH], dt.float32, tag="self1",
                                      name=f"self1_{s}")
                    nc.vector.tensor_scalar_mul(self1[:], xown_t[:], dinv2_t[:, :1])
                    ag1r = pool.tile([P, IN_CH], dt.float32, tag="ag1r",
                                     name=f"ag1r{s}")
                    nc.vector.tensor_add(out=ag1r[:], in0=ag1[:], in1=self1[:])
                    a1s = pool.tile([P, K1 * P], dt.bfloat16, tag="a1s",
                                    name=f"a1s{s}")
                    for m in range(K1):
                        tp = psum1.tile([P, P], dt.float32, tag="tp1",
                                        name=f"tp1_{s}_{m}")
                        nc.tensor.transpose(tp[:], ag1r[:, m * P:(m + 1) * P],
                                            ident[:])
                        nc.vector.tensor_copy(out=a1s[:, m * P:(m + 1) * P],
                                              in_=tp[:])

                    zp = psum1.tile([P, HID_CH], dt.float32, tag="zp",
                                    name=f"zp{s}")
                    for m in range(K1):
                        nc.tensor.matmul(
                            out=zp[:],
                            lhsT=a1s[:, m * P:(m + 1) * P],
                            rhs=w1_t[:, m * HID_CH:(m + 1) * HID_CH],
                            start=(m == 0),
                            stop=(m == K1 - 1),
                        )
                    z_t = pool.tile([P, HID_CH], dt.bfloat16, tag="z",
                                    name=f"z{s}")
                    if has_bias:
                        nc.vector.tensor_add(out=zp[:], in0=zp[:], in1=b1_t[:])
                    nc.scalar.activation(out=z_t[:], in_=zp[:],
                                         func=mybir.ActivationFunctionType.Relu)
                    nc.sync.dma_start(out=z_slice[s * P:(s + 1) * P, :], in_=z_t[:])

            # ---------------- AllGather z ----------------
            nc.gpsimd.collective_compute(
                "AllGather",
                mybir.AluOpType.bypass,
                replica_groups=[list(range(N_CORES))],
                ins=[z_slice.opt()],
                outs=[z_full.opt()],
            )
            if DEBUG_Z:
                nc.sync.dma_start(out=t_zdbg[:], in_=z_full[:])

            # ---------------- Layer 2 ----------------
            with tc.tile_pool(name="psum2", bufs=2, space="PSUM") as psum2:
                for s in range(SUP_PER_CORE):
                    zidx_t = pool.tile([P, B], dt.int32, tag="zidx",
                                       name=f"zidx{s}")
                    nc.sync.dma_start(out=zidx_t[:], in_=t_zidx[s])
                    g_t = bigpool.tile([P, B * HID_CH], dt.bfloat16, tag="g",
                                       name=f"g{s}")
                    for b in range(B):
                        nc.gpsimd.indirect_dma_start(
                            out=g_t[:, b * HID_CH:(b + 1) * HID_CH],
                            out_offset=None,
                            in_=z_full[:],
                            in_offset=bass.IndirectOffsetOnAxis(
                                ap=zidx_t[:, b:b + 1], axis=0
                            ),
                        )
                    st_t = bigpool.tile([P, B * P], dt.bfloat16, tag="st2",
                                        name=f"st2_{s}")
                    nc.sync.dma_start(out=st_t[:], in_=t_st[s])
                    ag2 = psum2.tile([P, HID_CH], dt.float32, tag="ag2",
                                     name=f"ag2_{s}")
                    for b in range(B):
                        nc.tensor.matmul(
                            out=ag2[:],
                            lhsT=st_t[:, b * P:(b + 1) * P],
                            rhs=g_t[:, b * HID_CH:(b + 1) * HID_CH],
                            start=(b == 0),
                            stop=(b == B - 1),
                        )
                    zown_t = pool.tile([P, HID_CH], dt.bfloat16, tag="zown",
                                       name=f"zown{s}")
                    nc.sync.dma_start(out=zown_t[:],
                                      in_=z_slice[s * P:(s + 1) * P, :])
                    dinv2_t2 = pool.tile([P, 1], dt.float32, tag="dinv2b",
                                         name=f"dinv2b{s}")
                    nc.sync.dma_start(out=dinv2_t2[:], in_=t_dinv2[s])
                    self2 = pool.tile([P, HID_CH], dt.float32, tag="self2",
                                      name=f"self2_{s}")
                    nc.vector.tensor_scalar_mul(self2[:], zown_t[:], dinv2_t2[:, :1])
                    ag2r = pool.tile([P, HID_CH], dt.float32, tag="ag2r",
                                     name=f"ag2r{s}")
                    nc.vector.tensor_add(out=ag2r[:], in0=ag2[:], in1=self2[:])
                    a2s = pool.tile([P, K2 * P], dt.bfloat16, tag="a2s",
                                    name=f"a2s{s}")
                    for m in range(K2):
                        tp = psum2.tile([P, P], dt.float32, tag="tp2",
                                        name=f"tp2_{s}_{m}")
                        nc.tensor.transpose(tp[:], ag2r[:, m * P:(m + 1) * P],
                                            ident[:])
                        nc.vector.tensor_copy(out=a2s[:, m * P:(m + 1) * P],
                                              in_=tp[:])

                    op = psum2.tile([P, HID_CH], dt.float32, tag="op",
                                    name=f"op{s}")
                    for m in range(K2):
                        nc.tensor.matmul(
                            out=op[:],
                            lhsT=a2s[:, m * P:(m + 1) * P],
                            rhs=w2_t[:, m * HID_CH:(m + 1) * HID_CH],
                            start=(m == 0),
                            stop=(m == K2 - 1),
                        )
                    o_t = pool.tile([P, HID_CH], dt.float32, tag="o",
                                    name=f"o{s}")
                    if has_bias:
                        nc.vector.tensor_add(out=o_t[:], in0=op[:], in1=b2_t[:])
                    else:
                        nc.vector.tensor_copy(out=o_t[:], in_=op[:])
                    nc.sync.dma_start(out=t_out[s * P:(s + 1) * P, :], in_=o_t[:])

    nc.compile()
    return nc


def kernel(x, edge_index, W1, b1, W2, b2):
    global LAST_RESULTS
    from concourse import bass_utils

    x = np.asarray(x, np.float32)
    edge_index = np.asarray(edge_index)
    W1 = np.asarray(W1, np.float32)
    b1 = np.asarray(b1, np.float32)
    W2 = np.asarray(W2, np.float32)
    b2 = np.asarray(b2, np.float32)

    prep = _preprocess(x, edge_index)
    B = prep["B"]
    has_bias = bool(np.any(b1) or np.any(b2))

    key = (B, has_bias, DEBUG_Z)
    if key not in _COMPILED:
        _COMPILED[key] = _build_program(B, has_bias)
    nc = _COMPILED[key]

    w1_in = np.ascontiguousarray(
        W1.astype(bf16).reshape(2, P, HID_CH)
    )
    w2_in = np.ascontiguousarray(
        W2.astype(bf16).reshape(4, P, HID_CH)
    )

    in_maps = []
    for c in range(N_CORES):
        s0, s1 = c * SUP_PER_CORE, (c + 1) * SUP_PER_CORE
        m = {
            "xe": np.ascontiguousarray(prep["xe"][s0:s1]),
            "st": np.ascontiguousarray(prep["st"][s0:s1]),
            "zidx": np.ascontiguousarray(prep["zidx"][s0:s1]),
            "xown": np.ascontiguousarray(prep["xown"][s0:s1]),
            "dinv2": np.ascontiguousarray(prep["dinv2"][s0:s1]),
            "w1": w1_in,
            "w2": w2_in,
        }
        if has_bias:
            m["b1b"] = np.tile(b1.astype(np.float32)[None, :], (P, 1))
            m["b2b"] = np.tile(b2.astype(np.float32)[None, :], (P, 1))
        in_maps.append(m)

    res = bass_utils.run_bass_kernel_spmd(
        nc, in_maps, core_ids=list(range(N_CORES)), trace=TRACE,
    )
    LAST_RESULTS = res

    out = np.concatenate([res.results[c]["out"] for c in range(N_CORES)], axis=0)
    return np.ascontiguousarray(out[:N_NODES]).astype(np.float32)


# revision 32
# speedup vs baseline: 1.0422x; 1.0422x over previous
"""Two-layer GCN (PyG GCNConv x2 + ReLU) on 8 Trainium2 NeuronCores via Bass.

Formulation: GCN aggregation is linear row-mixing, so for each layer
    conv(H) = A_hat @ H @ W + b      (A_hat includes self-loops, sym-norm)
and we aggregate FIRST, then matmul:
    z   = relu(A_hat @ x @ W1 + b1)
    out = A_hat @ z @ W2 + b2

Sharding: nodes split into 8 row-slabs (2560 padded rows each). Each core
aggregates + matmuls its own dst rows. The only communication is an
AllGather of z (bf16), split in two chunks so it overlaps layer-1 compute.

Aggregation: edges bucketed by 128-dst-node supertile, padded to B blocks
of 128 edges. Per block, gathered src rows G [128 edges, C] stream against
a host-built stationary selection matrix S^T [128 edges, 128 dst]:
    agg[dst, :] += S^T.T @ G       (PSUM accumulate, one matmul per block)
Row-major agg is PE-transposed into lhsT form for the @W matmul.

Layer 1 gathers are free (x is an input: the edge-ordered table xe is
host-built and streamed sequentially; self-loops included as edges).
Layer 2 gathers rows of all-gathered z via indirect DMA on GpSimd — the
hard bottleneck (~1.4us per 128-row block, POOL desc-gen serialized). To
hide it: self-loops are applied as a per-partition DVE term instead of
edges, and L2 runs in two passes (lo-half srcs right after the first AG
chunk, hi-half after the second) with lo partials staged in DRAM.
"""

import numpy as np
import ml_dtypes

N_NODES = 20000
IN_CH = 256
HID_CH = 512
N_CORES = 8
P = 128
NPAD = 20480
NSUP = NPAD // P                 # 160
SUP_PER_CORE = NSUP // N_CORES   # 20
ROWS_PER_CORE = NPAD // N_CORES  # 2560
HALF = ROWS_PER_CORE // 2        # 1280 rows per AG chunk per core

bf16 = ml_dtypes.bfloat16

TRACE = False
LAST_RESULTS = None

_COMPILED = {}


def _bucket(svals, dvals, wvals, nsup, cap):
    """Scatter edges (sorted by dst) into padded per-supertile slots."""
    sup = (dvals >> 7).astype(np.int64)
    cnt = np.bincount(sup, minlength=nsup)
    starts = np.zeros(nsup, np.int64)
    starts[1:] = np.cumsum(cnt)[:-1]
    slot = np.arange(len(dvals)) - starts[sup]
    gslot = sup * cap + slot
    esrc = np.zeros(nsup * cap, np.int64)
    esrc[gslot] = svals
    edstl = np.zeros(nsup * cap, np.int64)
    edstl[gslot] = dvals & 127
    ew = np.zeros(nsup * cap, np.float32)
    ew[gslot] = wvals
    return esrc, edstl, ew


def _st_blocks(edstl, ew, nsup, nblk):
    """Selection matrices in DMA layout [nsup, p, b*128+dstl]."""
    st = np.zeros((nsup * nblk * P, P), bf16)
    st[np.arange(nsup * nblk * P), edstl] = ew.astype(bf16)
    return (
        st.reshape(nsup, nblk, P, P)
        .transpose(0, 2, 1, 3)
        .reshape(nsup, P, nblk * P)
    )


def _preprocess(x, edge_index):
    src = edge_index[0].astype(np.int64)
    dst = edge_index[1].astype(np.int64)
    n = N_NODES

    deg = np.bincount(dst, minlength=n).astype(np.float32) + 1.0
    dinv = 1.0 / np.sqrt(deg)
    w = (dinv[src] * dinv[dst]).astype(np.float32)

    order = np.argsort(dst, kind="stable")
    s_src, s_dst, s_w = src[order], dst[order], w[order]

    # ---- layer 1: edges + self-loops (gathers are host-side, free) ----
    a_src = np.concatenate([s_src, np.arange(n)])
    a_dst = np.concatenate([s_dst, np.arange(n)])
    a_w = np.concatenate([s_w, dinv * dinv]).astype(np.float32)
    o1 = np.argsort(a_dst, kind="stable")
    a_src, a_dst, a_w = a_src[o1], a_dst[o1], a_w[o1]
    cnt1 = np.bincount(a_dst >> 7, minlength=NSUP)
    B1 = int(np.ceil(cnt1.max() / P))
    esrc1, edstl1, ew1 = _bucket(a_src, a_dst, a_w, NSUP, B1 * P)
    st1 = _st_blocks(edstl1, ew1, NSUP, B1)
    xbf = np.ascontiguousarray(x.astype(bf16))
    xe = (
        xbf[esrc1.reshape(NSUP, B1, P)]
        .transpose(0, 2, 1, 3)
        .reshape(NSUP, P, B1 * IN_CH)
    )

    # ---- layer 2: no self-loops; edges split by src half-slab ----
    half_pos = s_src % ROWS_PER_CORE
    lo_m = half_pos < HALF
    res = {"B1": B1, "st1": st1, "xe": xe}
    for name, mask in (("lo", lo_m), ("hi", ~lo_m)):
        ms, md, mw = s_src[mask], s_dst[mask], s_w[mask]
        cnt = np.bincount(md >> 7, minlength=NSUP)
        B = int(np.ceil(cnt.max() / P))
        esrc, edstl, ew = _bucket(ms, md, mw, NSUP, B * P)
        # z_lo/z_hi row layout: [rank, 1280] slabs
        zrow = (esrc // ROWS_PER_CORE) * HALF + (esrc % ROWS_PER_CORE) % HALF
        res[f"B2{name}"] = B
        res[f"st2{name}"] = _st_blocks(edstl, ew, NSUP, B)
        res[f"zidx{name}"] = (
            zrow.astype(np.int32).reshape(NSUP, B, P).transpose(0, 2, 1).copy()
        )

    dinv2 = np.zeros((NPAD,), np.float32)
    dinv2[:n] = dinv * dinv
    res["dinv2"] = dinv2.reshape(NSUP, P, 1)
    return res


def _build_program(B1, B2lo, B2hi, has_bias):
    import concourse.bass as bass
    import concourse.mybir as mybir
    import concourse.tile as tile
    from concourse.bacc import Bacc
    from concourse.masks import make_identity

    dt = mybir.dt
    nc = Bacc("TRN2", target_bir_lowering=False, debug=False, num_devices=N_CORES)

    t_xe = nc.dram_tensor("xe", [SUP_PER_CORE, P, B1 * IN_CH], dt.bfloat16,
                          kind="ExternalInput")
    t_st1 = nc.dram_tensor("st1", [SUP_PER_CORE, P, B1 * P], dt.bfloat16,
                           kind="ExternalInput")
    t_st2lo = nc.dram_tensor("st2lo", [SUP_PER_CORE, P, B2lo * P], dt.bfloat16,
                             kind="ExternalInput")
    t_st2hi = nc.dram_tensor("st2hi", [SUP_PER_CORE, P, B2hi * P], dt.bfloat16,
                             kind="ExternalInput")
    t_zidxlo = nc.dram_tensor("zidxlo", [SUP_PER_CORE, P, B2lo], dt.int32,
                              kind="ExternalInput")
    t_zidxhi = nc.dram_tensor("zidxhi", [SUP_PER_CORE, P, B2hi], dt.int32,
                              kind="ExternalInput")
    t_dinv2 = nc.dram_tensor("dinv2", [SUP_PER_CORE, P, 1], dt.float32,
                             kind="ExternalInput")
    t_w1 = nc.dram_tensor("w1", [2, P, HID_CH], dt.bfloat16, kind="ExternalInput")
    t_w2 = nc.dram_tensor("w2", [4, P, HID_CH], dt.bfloat16, kind="ExternalInput")
    if has_bias:
        t_b1 = nc.dram_tensor("b1b", [P, HID_CH], dt.float32, kind="ExternalInput")
        t_b2 = nc.dram_tensor("b2b", [P, HID_CH], dt.float32, kind="ExternalInput")
    t_out = nc.dram_tensor("out", [ROWS_PER_CORE, HID_CH], dt.float32,
                           kind="ExternalOutput")

    K1 = IN_CH // P
    K2 = HID_CH // P
    SPH = SUP_PER_CORE // 2  # supertiles per AG chunk

    with tile.TileContext(nc) as tc:
        with (
            tc.tile_pool(name="dram", bufs=1, space="DRAM") as dram,
            tc.tile_pool(name="const", bufs=1) as cpool,
            tc.tile_pool(name="work", bufs=4) as pool,
            tc.tile_pool(name="big", bufs=3) as bigpool,
        ):
            z_slice = dram.tile([ROWS_PER_CORE, HID_CH], dt.bfloat16,
                                name="z_slice")
            z_lo = dram.tile([N_CORES * HALF, HID_CH], dt.bfloat16, name="z_lo",
                             addr_space="Shared")
            z_hi = dram.tile([N_CORES * HALF, HID_CH], dt.bfloat16, name="z_hi",
                             addr_space="Shared")
            partial = dram.tile([SUP_PER_CORE, P, HID_CH], dt.bfloat16,
                                name="partial")

            w1_t = cpool.tile([P, K1 * HID_CH], dt.bfloat16, name="w1_t")
            for m in range(K1):
                nc.sync.dma_start(out=w1_t[:, m * HID_CH:(m + 1) * HID_CH],
                                  in_=t_w1[m])
            w2_t = cpool.tile([P, K2 * HID_CH], dt.bfloat16, name="w2_t")
            for m in range(K2):
                nc.sync.dma_start(out=w2_t[:, m * HID_CH:(m + 1) * HID_CH],
                                  in_=t_w2[m])
            if has_bias:
                b1_t = cpool.tile([P, HID_CH], dt.float32, name="b1_t")
                nc.sync.dma_start(out=b1_t[:], in_=t_b1[:])
                b2_t = cpool.tile([P, HID_CH], dt.float32, name="b2_t")
                nc.sync.dma_start(out=b2_t[:], in_=t_b2[:])
            ident = cpool.tile([P, P], dt.float32, name="ident")
            make_identity(nc, ident[:])

            # ---------------- Layer 1 (+ chunked AllGather) ----------------
            with tc.tile_pool(name="psum1", bufs=2, space="PSUM") as psum1:
                for s in range(SUP_PER_CORE):
                    xe_t = bigpool.tile([P, B1 * IN_CH], dt.bfloat16, tag="xe",
                                        name=f"xe{s}")
                    nc.sync.dma_start(out=xe_t[:], in_=t_xe[s])
                    st_t = bigpool.tile([P, B1 * P], dt.bfloat16, tag="st1",
                                        name=f"st{s}")
                    nc.sync.dma_start(out=st_t[:], in_=t_st1[s])

                    ag1 = psum1.tile([P, IN_CH], dt.float32, tag="ag1",
                                     name=f"ag1_{s}")
                    for b in range(B1):
                        nc.tensor.matmul(
                            out=ag1[:],
                            lhsT=st_t[:, b * P:(b + 1) * P],
                            rhs=xe_t[:, b * IN_CH:(b + 1) * IN_CH],
                            start=(b == 0),
                            stop=(b == B1 - 1),
                        )
                    ag1r = pool.tile([P, IN_CH], dt.float32, tag="ag1r",
                                     name=f"ag1r{s}")
                    nc.vector.tensor_copy(out=ag1r[:], in_=ag1[:])
                    a1s = pool.tile([P, K1 * P], dt.bfloat16, tag="a1s",
                                    name=f"a1s{s}")
                    for m in range(K1):
                        tp = psum1.tile([P, P], dt.float32, tag="tp1",
                                        name=f"tp1_{s}_{m}")
                        nc.tensor.transpose(tp[:], ag1r[:, m * P:(m + 1) * P],
                                            ident[:])
                        nc.vector.tensor_copy(out=a1s[:, m * P:(m + 1) * P],
                                              in_=tp[:])
                    zp = psum1.tile([P, HID_CH], dt.float32, tag="zp",
                                    name=f"zp{s}")
                    for m in range(K1):
                        nc.tensor.matmul(
                            out=zp[:],
                            lhsT=a1s[:, m * P:(m + 1) * P],
                            rhs=w1_t[:, m * HID_CH:(m + 1) * HID_CH],
                            start=(m == 0),
                            stop=(m == K1 - 1),
                        )
                    z_t = pool.tile([P, HID_CH], dt.bfloat16, tag="z",
                                    name=f"z{s}")
                    if has_bias:
                        nc.vector.tensor_add(out=zp[:], in0=zp[:], in1=b1_t[:])
                    nc.scalar.activation(out=z_t[:], in_=zp[:],
                                         func=mybir.ActivationFunctionType.Relu)
                    nc.sync.dma_start(out=z_slice[s * P:(s + 1) * P, :],
                                      in_=z_t[:])
                    if s == SPH - 1:
                        nc.gpsimd.collective_compute(
                            "AllGather", mybir.AluOpType.bypass,
                            replica_groups=[list(range(N_CORES))],
                            ins=[z_slice[0:HALF, :]],
                            outs=[z_lo.opt()],
                        )
                nc.gpsimd.collective_compute(
                    "AllGather", mybir.AluOpType.bypass,
                    replica_groups=[list(range(N_CORES))],
                    ins=[z_slice[HALF:ROWS_PER_CORE, :]],
                    outs=[z_hi.opt()],
                )

            # ---------------- Layer 2 pass A: lo-half srcs ----------------
            with tc.tile_pool(name="psumA", bufs=2, space="PSUM") as psumA:
                for s in range(SUP_PER_CORE):
                    zidx_t = pool.tile([P, B2lo], dt.int32, tag="zidxlo",
                                       name=f"zidxlo{s}")
                    nc.sync.dma_start(out=zidx_t[:], in_=t_zidxlo[s])
                    g_t = bigpool.tile([P, B2lo * HID_CH], dt.bfloat16, tag="glo",
                                       name=f"glo{s}")
                    for b in range(B2lo):
                        nc.gpsimd.indirect_dma_start(
                            out=g_t[:, b * HID_CH:(b + 1) * HID_CH],
                            out_offset=None,
                            in_=z_lo[:],
                            in_offset=bass.IndirectOffsetOnAxis(
                                ap=zidx_t[:, b:b + 1], axis=0),
                        )
                    st_t = bigpool.tile([P, B2lo * P], dt.bfloat16, tag="st2lo",
                                        name=f"st2lo{s}")
                    nc.sync.dma_start(out=st_t[:], in_=t_st2lo[s])
                    agA = psumA.tile([P, HID_CH], dt.float32, tag="agA",
                                     name=f"agA_{s}")
                    for b in range(B2lo):
                        nc.tensor.matmul(
                            out=agA[:],
                            lhsT=st_t[:, b * P:(b + 1) * P],
                            rhs=g_t[:, b * HID_CH:(b + 1) * HID_CH],
                            start=(b == 0),
                            stop=(b == B2lo - 1),
                        )
                    pl_t = pool.tile([P, HID_CH], dt.bfloat16, tag="pl",
                                     name=f"pl{s}")
                    nc.vector.tensor_copy(out=pl_t[:], in_=agA[:])
                    nc.sync.dma_start(out=partial[s], in_=pl_t[:])

            # ---------------- Layer 2 pass B: hi-half + finish ----------------
            with tc.tile_pool(name="psumB", bufs=2, space="PSUM") as psumB:
                for s in range(SUP_PER_CORE):
                    zidx_t = pool.tile([P, B2hi], dt.int32, tag="zidxhi",
                                       name=f"zidxhi{s}")
                    nc.sync.dma_start(out=zidx_t[:], in_=t_zidxhi[s])
                    g_t = bigpool.tile([P, B2hi * HID_CH], dt.bfloat16, tag="ghi",
                                       name=f"ghi{s}")
                    for b in range(B2hi):
                        nc.gpsimd.indirect_dma_start(
                            out=g_t[:, b * HID_CH:(b + 1) * HID_CH],
                            out_offset=None,
                            in_=z_hi[:],
                            in_offset=bass.IndirectOffsetOnAxis(
                                ap=zidx_t[:, b:b + 1], axis=0),
                        )
                    st_t = bigpool.tile([P, B2hi * P], dt.bfloat16, tag="st2hi",
                                        name=f"st2hi{s}")
                    nc.sync.dma_start(out=st_t[:], in_=t_st2hi[s])
                    agB = psumB.tile([P, HID_CH], dt.float32, tag="agB",
                                     name=f"agB_{s}")
                    for b in range(B2hi):
                        nc.tensor.matmul(
                            out=agB[:],
                            lhsT=st_t[:, b * P:(b + 1) * P],
                            rhs=g_t[:, b * HID_CH:(b + 1) * HID_CH],
                            start=(b == 0),
                            stop=(b == B2hi - 1),
                        )
                    # agg2 = agB + partial_lo + dinv2 * z_own
                    pl_t = pool.tile([P, HID_CH], dt.bfloat16, tag="plb",
                                     name=f"plb{s}")
                    nc.sync.dma_start(out=pl_t[:], in_=partial[s])
                    zown_t = pool.tile([P, HID_CH], dt.bfloat16, tag="zown",
                                       name=f"zown{s}")
                    nc.sync.dma_start(out=zown_t[:],
                                      in_=z_slice[s * P:(s + 1) * P, :])
                    dinv2_t = pool.tile([P, 1], dt.float32, tag="dinv2",
                                        name=f"dinv2{s}")
                    nc.sync.dma_start(out=dinv2_t[:], in_=t_dinv2[s])
                    ag2r = pool.tile([P, HID_CH], dt.float32, tag="ag2r",
                                     name=f"ag2r{s}")
                    nc.vector.scalar_tensor_tensor(
                        out=ag2r[:], in0=zown_t[:], scalar=dinv2_t[:, :1],
                        in1=agB[:], op0=mybir.AluOpType.mult,
                        op1=mybir.AluOpType.add,
                    )
                    nc.vector.tensor_add(out=ag2r[:], in0=ag2r[:], in1=pl_t[:])

                    a2s = pool.tile([P, K2 * P], dt.bfloat16, tag="a2s",
                                    name=f"a2s{s}")
                    for m in range(K2):
                        tp = psumB.tile([P, P], dt.float32, tag="tp2",
                                        name=f"tp2_{s}_{m}")
                        nc.tensor.transpose(tp[:], ag2r[:, m * P:(m + 1) * P],
                                            ident[:])
                        nc.vector.tensor_copy(out=a2s[:, m * P:(m + 1) * P],
                                              in_=tp[:])
                    op = psumB.tile([P, HID_CH], dt.float32, tag="op",
                                    name=f"op{s}")
                    for m in range(K2):
                        nc.tensor.matmul(
                            out=op[:],
                            lhsT=a2s[:, m * P:(m + 1) * P],
                            rhs=w2_t[:, m * HID_CH:(m + 1) * HID_CH],
                            start=(m == 0),
                            stop=(m == K2 - 1),
                        )
                    o_t = pool.tile([P, HID_CH], dt.float32, tag="o",
                                    name=f"o{s}")
                    if has_bias:
                        nc.vector.tensor_add(out=o_t[:], in0=op[:], in1=b2_t[:])
                    else:
                        nc.vector.tensor_copy(out=o_t[:], in_=op[:])
                    nc.sync.dma_start(out=t_out[s * P:(s + 1) * P, :], in_=o_t[:])

    nc.compile()
    return nc


def kernel(x, edge_index, W1, b1, W2, b2):
    global LAST_RESULTS
    from concourse import bass_utils

    x = np.asarray(x, np.float32)
    edge_index = np.asarray(edge_index)
    W1 = np.asarray(W1, np.float32)
    b1 = np.asarray(b1, np.float32)
    W2 = np.asarray(W2, np.float32)
    b2 = np.asarray(b2, np.float32)

    prep = _preprocess(x, edge_index)
    B1, B2lo, B2hi = prep["B1"], prep["B2lo"], prep["B2hi"]
    has_bias = bool(np.any(b1) or np.any(b2))

    key = (B1, B2lo, B2hi, has_bias)
    if key not in _COMPILED:
        _COMPILED[key] = _build_program(B1, B2lo, B2hi, has_bias)
    nc = _COMPILED[key]

    w1_in = np.ascontiguousarray(W1.astype(bf16).reshape(2, P, HID_CH))
    w2_in = np.ascontiguousarray(W2.astype(bf16).reshape(4, P, HID_CH))

    in_maps = []
    for c in range(N_CORES):
        s0, s1 = c * SUP_PER_CORE, (c + 1) * SUP_PER_CORE
        m = {
            "xe": np.ascontiguousarray(prep["xe"][s0:s1]),
            "st1": np.ascontiguousarray(prep["st1"][s0:s1]),
            "st2lo": np.ascontiguousarray(prep["st2lo"][s0:s1]),
            "st2hi": np.ascontiguousarray(prep["st2hi"][s0:s1]),
            "zidxlo": np.ascontiguousarray(prep["zidxlo"][s0:s1]),
            "zidxhi": np.ascontiguousarray(prep["zidxhi"][s0:s1]),
            "dinv2": np.ascontiguousarray(prep["dinv2"][s0:s1]),
            "w1": w1_in,
            "w2": w2_in,
        }
        if has_bias:
            m["b1b"] = np.tile(b1.astype(np.float32)[None, :], (P, 1))
            m["b2b"] = np.tile(b2.astype(np.float32)[None, :], (P, 1))
        in_maps.append(m)

    res = bass_utils.run_bass_kernel_spmd(
        nc, in_maps, core_ids=list(range(N_CORES)), trace=TRACE,
    )
    LAST_RESULTS = res

    out = np.concatenate([res.results[c]["out"] for c in range(N_CORES)], axis=0)
    return np.ascontiguousarray(out[:N_NODES]).astype(np.float32)


# revision 37
# speedup vs baseline: 1.1749x; 1.1274x over previous
"""Two-layer GCN (PyG GCNConv x2 + ReLU) on 8 Trainium2 NeuronCores via Bass.

Formulation: GCN aggregation is linear row-mixing, so for each layer
    conv(H) = A_hat @ H @ W + b      (A_hat includes self-loops, sym-norm)
and we aggregate FIRST, then matmul:
    z   = relu(A_hat @ x @ W1 + b1)
    out = A_hat @ z @ W2 + b2

Sharding: nodes split into 8 row-slabs (2560 padded rows each). Each core
aggregates + matmuls its own dst rows. The only communication is an
AllGather of z (bf16), split in two chunks so it overlaps layer-1 compute.

Aggregation: edges bucketed by 128-dst-node supertile, padded to B blocks
of 128 edges. Per block, gathered src rows G [128 edges, C] stream against
a host-built stationary selection matrix S^T [128 edges, 128 dst]:
    agg[dst, :] += S^T.T @ G       (PSUM accumulate, one matmul per block)
Row-major agg is PE-transposed into lhsT form for the @W matmul.

Layer 1 gathers are free (x is an input: the edge-ordered table xe is
host-built and streamed sequentially; self-loops included as edges).
Layer 2 gathers rows of all-gathered z via indirect DMA on GpSimd — the
hard bottleneck (~1.4us per 128-row block, POOL desc-gen serialized). To
hide it: self-loops are applied as a per-partition DVE term instead of
edges, and L2 runs in two passes (lo-half srcs right after the first AG
chunk, hi-half after the second) with lo partials staged in DRAM.
"""

import numpy as np
import ml_dtypes

N_NODES = 20000
IN_CH = 256
HID_CH = 512
N_CORES = 8
P = 128
NPAD = 20480
NSUP = NPAD // P                 # 160
SUP_PER_CORE = NSUP // N_CORES   # 20
ROWS_PER_CORE = NPAD // N_CORES  # 2560
HALF = ROWS_PER_CORE // 2        # 1280 rows per AG chunk per core

bf16 = ml_dtypes.bfloat16

TRACE = False
LAST_RESULTS = None

_COMPILED = {}


def _bucket(svals, dvals, wvals, nsup, cap):
    """Scatter edges (sorted by dst) into padded per-supertile slots."""
    sup = (dvals >> 7).astype(np.int64)
    cnt = np.bincount(sup, minlength=nsup)
    starts = np.zeros(nsup, np.int64)
    starts[1:] = np.cumsum(cnt)[:-1]
    slot = np.arange(len(dvals)) - starts[sup]
    gslot = sup * cap + slot
    esrc = np.zeros(nsup * cap, np.int64)
    esrc[gslot] = svals
    edstl = np.zeros(nsup * cap, np.int64)
    edstl[gslot] = dvals & 127
    ew = np.zeros(nsup * cap, np.float32)
    ew[gslot] = wvals
    return esrc, edstl, ew


def _st_blocks(edstl, ew, nsup, nblk):
    """Selection matrices in DMA layout [nsup, p, b*128+dstl]."""
    st = np.zeros((nsup * nblk * P, P), bf16)
    st[np.arange(nsup * nblk * P), edstl] = ew.astype(bf16)
    return (
        st.reshape(nsup, nblk, P, P)
        .transpose(0, 2, 1, 3)
        .reshape(nsup, P, nblk * P)
    )


def _preprocess(x, edge_index):
    src = edge_index[0].astype(np.int64)
    dst = edge_index[1].astype(np.int64)
    n = N_NODES

    deg = np.bincount(dst, minlength=n).astype(np.float32) + 1.0
    dinv = 1.0 / np.sqrt(deg)
    w = (dinv[src] * dinv[dst]).astype(np.float32)

    order = np.argsort(dst, kind="stable")
    s_src, s_dst, s_w = src[order], dst[order], w[order]

    # ---- layer 1: edges + self-loops (gathers are host-side, free) ----
    a_src = np.concatenate([s_src, np.arange(n)])
    a_dst = np.concatenate([s_dst, np.arange(n)])
    a_w = np.concatenate([s_w, dinv * dinv]).astype(np.float32)
    o1 = np.argsort(a_dst, kind="stable")
    a_src, a_dst, a_w = a_src[o1], a_dst[o1], a_w[o1]
    cnt1 = np.bincount(a_dst >> 7, minlength=NSUP)
    B1 = int(np.ceil(cnt1.max() / P))
    esrc1, edstl1, ew1 = _bucket(a_src, a_dst, a_w, NSUP, B1 * P)
    st1 = _st_blocks(edstl1, ew1, NSUP, B1)
    xbf = np.ascontiguousarray(x.astype(bf16))
    xe = (
        xbf[esrc1.reshape(NSUP, B1, P)]
        .transpose(0, 2, 1, 3)
        .reshape(NSUP, P, B1 * IN_CH)
    )

    # ---- layer 2: no self-loops; edges split by src half-slab ----
    half_pos = s_src % ROWS_PER_CORE
    lo_m = half_pos < HALF
    res = {"B1": B1, "st1": st1, "xe": xe}
    for name, mask in (("lo", lo_m), ("hi", ~lo_m)):
        ms, md, mw = s_src[mask], s_dst[mask], s_w[mask]
        cnt = np.bincount(md >> 7, minlength=NSUP)
        B = int(np.ceil(cnt.max() / P))
        esrc, edstl, ew = _bucket(ms, md, mw, NSUP, B * P)
        # z_lo/z_hi row layout: [rank, 1280] slabs
        zrow = (esrc // ROWS_PER_CORE) * HALF + (esrc % ROWS_PER_CORE) % HALF
        res[f"B2{name}"] = B
        res[f"st2{name}"] = _st_blocks(edstl, ew, NSUP, B)
        res[f"zidx{name}"] = (
            zrow.astype(np.int32).reshape(NSUP, B, P).transpose(0, 2, 1).copy()
        )

    dinv2 = np.zeros((NPAD,), np.float32)
    dinv2[:n] = dinv * dinv
    res["dinv2"] = dinv2.reshape(NSUP, P, 1)
    return res


def _build_program(B1, B2lo, B2hi, has_bias):
    import concourse.bass as bass
    import concourse.mybir as mybir
    import concourse.tile as tile
    from concourse.bacc import Bacc
    from concourse.masks import make_identity

    dt = mybir.dt
    nc = Bacc("TRN2", target_bir_lowering=False, debug=False, num_devices=N_CORES)

    t_xe = nc.dram_tensor("xe", [SUP_PER_CORE, P, B1 * IN_CH], dt.bfloat16,
                          kind="ExternalInput")
    t_st1 = nc.dram_tensor("st1", [SUP_PER_CORE, P, B1 * P], dt.bfloat16,
                           kind="ExternalInput")
    t_st2lo = nc.dram_tensor("st2lo", [SUP_PER_CORE, P, B2lo * P], dt.bfloat16,
                             kind="ExternalInput")
    t_st2hi = nc.dram_tensor("st2hi", [SUP_PER_CORE, P, B2hi * P], dt.bfloat16,
                             kind="ExternalInput")
    t_zidxlo = nc.dram_tensor("zidxlo", [SUP_PER_CORE, P, B2lo], dt.int32,
                              kind="ExternalInput")
    t_zidxhi = nc.dram_tensor("zidxhi", [SUP_PER_CORE, P, B2hi], dt.int32,
                              kind="ExternalInput")
    t_dinv2 = nc.dram_tensor("dinv2", [SUP_PER_CORE, P, 1], dt.float32,
                             kind="ExternalInput")
    t_w1 = nc.dram_tensor("w1", [2, P, HID_CH], dt.bfloat16, kind="ExternalInput")
    t_w2 = nc.dram_tensor("w2", [4, P, HID_CH], dt.bfloat16, kind="ExternalInput")
    if has_bias:
        t_b1 = nc.dram_tensor("b1b", [P, HID_CH], dt.float32, kind="ExternalInput")
        t_b2 = nc.dram_tensor("b2b", [P, HID_CH], dt.float32, kind="ExternalInput")
    t_out = nc.dram_tensor("out", [ROWS_PER_CORE, HID_CH], dt.float32,
                           kind="ExternalOutput")

    K1 = IN_CH // P
    K2 = HID_CH // P
    SPH = SUP_PER_CORE // 2  # supertiles per AG chunk

    with tile.TileContext(nc) as tc:
        with (
            tc.tile_pool(name="dram", bufs=1, space="DRAM") as dram,
            tc.tile_pool(name="const", bufs=1) as cpool,
            tc.tile_pool(name="work", bufs=4) as pool,
            tc.tile_pool(name="big", bufs=3) as bigpool,
        ):
            z_slice = dram.tile([ROWS_PER_CORE, HID_CH], dt.bfloat16,
                                name="z_slice")
            z_lo = dram.tile([N_CORES * HALF, HID_CH], dt.bfloat16, name="z_lo",
                             addr_space="Shared")
            z_hi = dram.tile([N_CORES * HALF, HID_CH], dt.bfloat16, name="z_hi",
                             addr_space="Shared")
            partial = dram.tile([SUP_PER_CORE, P, HID_CH], dt.bfloat16,
                                name="partial")

            w1_t = cpool.tile([P, K1 * HID_CH], dt.bfloat16, name="w1_t")
            for m in range(K1):
                nc.sync.dma_start(out=w1_t[:, m * HID_CH:(m + 1) * HID_CH],
                                  in_=t_w1[m])
            w2_t = cpool.tile([P, K2 * HID_CH], dt.bfloat16, name="w2_t")
            for m in range(K2):
                nc.sync.dma_start(out=w2_t[:, m * HID_CH:(m + 1) * HID_CH],
                                  in_=t_w2[m])
            if has_bias:
                b1_t = cpool.tile([P, HID_CH], dt.float32, name="b1_t")
                nc.sync.dma_start(out=b1_t[:], in_=t_b1[:])
                b2_t = cpool.tile([P, HID_CH], dt.float32, name="b2_t")
                nc.sync.dma_start(out=b2_t[:], in_=t_b2[:])
            ident = cpool.tile([P, P], dt.float32, name="ident")
            make_identity(nc, ident[:])

            # ---------------- Layer 1 (+ chunked AllGather) ----------------
            with tc.tile_pool(name="psum1", bufs=2, space="PSUM") as psum1:
                for s in range(SUP_PER_CORE):
                    xe_t = bigpool.tile([P, B1 * IN_CH], dt.bfloat16, tag="xe",
                                        name=f"xe{s}")
                    nc.sync.dma_start(out=xe_t[:], in_=t_xe[s])
                    st_t = bigpool.tile([P, B1 * P], dt.bfloat16, tag="st1",
                                        name=f"st{s}")
                    nc.sync.dma_start(out=st_t[:], in_=t_st1[s])

                    ag1 = psum1.tile([P, IN_CH], dt.float32, tag="ag1",
                                     name=f"ag1_{s}")
                    for b in range(B1):
                        nc.tensor.matmul(
                            out=ag1[:],
                            lhsT=st_t[:, b * P:(b + 1) * P],
                            rhs=xe_t[:, b * IN_CH:(b + 1) * IN_CH],
                            start=(b == 0),
                            stop=(b == B1 - 1),
                        )
                    ag1r = pool.tile([P, IN_CH], dt.float32, tag="ag1r",
                                     name=f"ag1r{s}")
                    nc.vector.tensor_copy(out=ag1r[:], in_=ag1[:])
                    a1s = pool.tile([P, K1 * P], dt.bfloat16, tag="a1s",
                                    name=f"a1s{s}")
                    for m in range(K1):
                        tp = psum1.tile([P, P], dt.float32, tag="tp1",
                                        name=f"tp1_{s}_{m}")
                        nc.tensor.transpose(tp[:], ag1r[:, m * P:(m + 1) * P],
                                            ident[:])
                        nc.vector.tensor_copy(out=a1s[:, m * P:(m + 1) * P],
                                              in_=tp[:])
                    zp = psum1.tile([P, HID_CH], dt.float32, tag="zp",
                                    name=f"zp{s}")
                    for m in range(K1):
                        nc.tensor.matmul(
                            out=zp[:],
                            lhsT=a1s[:, m * P:(m + 1) * P],
                            rhs=w1_t[:, m * HID_CH:(m + 1) * HID_CH],
                            start=(m == 0),
                            stop=(m == K1 - 1),
                        )
                    z_t = pool.tile([P, HID_CH], dt.bfloat16, tag="z",
                                    name=f"z{s}")
                    if has_bias:
                        nc.vector.tensor_add(out=zp[:], in0=zp[:], in1=b1_t[:])
                    nc.scalar.activation(out=z_t[:], in_=zp[:],
                                         func=mybir.ActivationFunctionType.Relu)
                    nc.sync.dma_start(out=z_slice[s * P:(s + 1) * P, :],
                                      in_=z_t[:])
                    if s == SPH - 1:
                        nc.gpsimd.collective_compute(
                            "AllGather", mybir.AluOpType.bypass,
                            replica_groups=[list(range(N_CORES))],
                            ins=[z_slice[0:HALF, :]],
                            outs=[z_lo.opt()],
                        )
                nc.gpsimd.collective_compute(
                    "AllGather", mybir.AluOpType.bypass,
                    replica_groups=[list(range(N_CORES))],
                    ins=[z_slice[HALF:ROWS_PER_CORE, :]],
                    outs=[z_hi.opt()],
                )

            # ---------------- Layer 2 pass A: lo-half srcs ----------------
            with tc.tile_pool(name="psumA", bufs=2, space="PSUM") as psumA:
                zidxlo_all = cpool.tile([P, SUP_PER_CORE * B2lo], dt.int32,
                                        name="zidxlo_all")
                nc.sync.dma_start(
                    out=zidxlo_all[:].rearrange("p (s b) -> p s b",
                                                s=SUP_PER_CORE),
                    in_=t_zidxlo[:].rearrange("s p b -> p s b"))
                for s in range(SUP_PER_CORE):
                    g_t = bigpool.tile([P, B2lo * HID_CH], dt.bfloat16, tag="glo",
                                       name=f"glo{s}")
                    for b in range(B2lo):
                        nc.gpsimd.indirect_dma_start(
                            out=g_t[:, b * HID_CH:(b + 1) * HID_CH],
                            out_offset=None,
                            in_=z_lo[:],
                            in_offset=bass.IndirectOffsetOnAxis(
                                ap=zidxlo_all[:, s * B2lo + b:s * B2lo + b + 1],
                                axis=0),
                        )
                    st_t = bigpool.tile([P, B2lo * P], dt.bfloat16, tag="st2lo",
                                        name=f"st2lo{s}")
                    nc.sync.dma_start(out=st_t[:], in_=t_st2lo[s])
                    agA = psumA.tile([P, HID_CH], dt.float32, tag="agA",
                                     name=f"agA_{s}")
                    for b in range(B2lo):
                        nc.tensor.matmul(
                            out=agA[:],
                            lhsT=st_t[:, b * P:(b + 1) * P],
                            rhs=g_t[:, b * HID_CH:(b + 1) * HID_CH],
                            start=(b == 0),
                            stop=(b == B2lo - 1),
                        )
                    pl_t = pool.tile([P, HID_CH], dt.bfloat16, tag="pl",
                                     name=f"pl{s}")
                    nc.vector.tensor_copy(out=pl_t[:], in_=agA[:])
                    nc.sync.dma_start(out=partial[s], in_=pl_t[:])

            # ---------------- Layer 2 pass B: hi-half + finish ----------------
            with tc.tile_pool(name="psumB", bufs=2, space="PSUM") as psumB:
                zidxhi_all = cpool.tile([P, SUP_PER_CORE * B2hi], dt.int32,
                                        name="zidxhi_all")
                nc.sync.dma_start(
                    out=zidxhi_all[:].rearrange("p (s b) -> p s b",
                                                s=SUP_PER_CORE),
                    in_=t_zidxhi[:].rearrange("s p b -> p s b"))
                for s in range(SUP_PER_CORE):
                    g_t = bigpool.tile([P, B2hi * HID_CH], dt.bfloat16, tag="ghi",
                                       name=f"ghi{s}")
                    for b in range(B2hi):
                        nc.gpsimd.indirect_dma_start(
                            out=g_t[:, b * HID_CH:(b + 1) * HID_CH],
                            out_offset=None,
                            in_=z_hi[:],
                            in_offset=bass.IndirectOffsetOnAxis(
                                ap=zidxhi_all[:, s * B2hi + b:s * B2hi + b + 1],
                                axis=0),
                        )
                    st_t = bigpool.tile([P, B2hi * P], dt.bfloat16, tag="st2hi",
                                        name=f"st2hi{s}")
                    nc.sync.dma_start(out=st_t[:], in_=t_st2hi[s])
                    agB = psumB.tile([P, HID_CH], dt.float32, tag="agB",
                                     name=f"agB_{s}")
                    for b in range(B2hi):
                        nc.tensor.matmul(
                            out=agB[:],
                            lhsT=st_t[:, b * P:(b + 1) * P],
                            rhs=g_t[:, b * HID_CH:(b + 1) * HID_CH],
                            start=(b == 0),
                            stop=(b == B2hi - 1),
                        )
                    # agg2 = agB + partial_lo + dinv2 * z_own
                    pl_t = pool.tile([P, HID_CH], dt.bfloat16, tag="plb",
                                     name=f"plb{s}")
                    nc.sync.dma_start(out=pl_t[:], in_=partial[s])
                    zown_t = pool.tile([P, HID_CH], dt.bfloat16, tag="zown",
                                       name=f"zown{s}")
                    nc.sync.dma_start(out=zown_t[:],
                                      in_=z_slice[s * P:(s + 1) * P, :])
                    dinv2_t = pool.tile([P, 1], dt.float32, tag="dinv2",
                                        name=f"dinv2{s}")
                    nc.sync.dma_start(out=dinv2_t[:], in_=t_dinv2[s])
                    ag2r = pool.tile([P, HID_CH], dt.float32, tag="ag2r",
                                     name=f"ag2r{s}")
                    nc.vector.scalar_tensor_tensor(
                        out=ag2r[:], in0=zown_t[:], scalar=dinv2_t[:, :1],
                        in1=agB[:], op0=mybir.AluOpType.mult,
                        op1=mybir.AluOpType.add,
                    )
                    nc.vector.tensor_add(out=ag2r[:], in0=ag2r[:], in1=pl_t[:])

                    a2s = pool.tile([P, K2 * P], dt.bfloat16, tag="a2s",
                                    name=f"a2s{s}")
                    for m in range(K2):
                        tp = psumB.tile([P, P], dt.float32, tag="tp2",
                                        name=f"tp2_{s}_{m}")
                        nc.tensor.transpose(tp[:], ag2r[:, m * P:(m + 1) * P],
                                            ident[:])
                        nc.vector.tensor_copy(out=a2s[:, m * P:(m + 1) * P],
                                              in_=tp[:])
                    op = psumB.tile([P, HID_CH], dt.float32, tag="op",
                                    name=f"op{s}")
                    for m in range(K2):
                        nc.tensor.matmul(
                            out=op[:],
                            lhsT=a2s[:, m * P:(m + 1) * P],
                            rhs=w2_t[:, m * HID_CH:(m + 1) * HID_CH],
                            start=(m == 0),
                            stop=(m == K2 - 1),
                        )
                    o_t = pool.tile([P, HID_CH], dt.float32, tag="o",
                                    name=f"o{s}")
                    if has_bias:
                        nc.vector.tensor_add(out=o_t[:], in0=op[:], in1=b2_t[:])
                    else:
                        nc.vector.tensor_copy(out=o_t[:], in_=op[:])
                    nc.sync.dma_start(out=t_out[s * P:(s + 1) * P, :], in_=o_t[:])

    nc.compile()
    return nc


def kernel(x, edge_index, W1, b1, W2, b2):
    global LAST_RESULTS
    from concourse import bass_utils

    x = np.asarray(x, np.float32)
    edge_index = np.asarray(edge_index)
    W1 = np.asarray(W1, np.float32)
    b1 = np.asarray(b1, np.float32)
    W2 = np.asarray(W2, np.float32)
    b2 = np.asarray(b2, np.float32)

    prep = _preprocess(x, edge_index)
    B1, B2lo, B2hi = prep["B1"], prep["B2lo"], prep["B2hi"]
    has_bias = bool(np.any(b1) or np.any(b2))

    key = (B1, B2lo, B2hi, has_bias)
    if key not in _COMPILED:
        _COMPILED[key] = _build_program(B1, B2lo, B2hi, has_bias)
    nc = _COMPILED[key]

    w1_in = np.ascontiguousarray(W1.astype(bf16).reshape(2, P, HID_CH))
    w2_in = np.ascontiguousarray(W2.astype(bf16).reshape(4, P, HID_CH))

    in_maps = []
    for c in range(N_CORES):
        s0, s1 = c * SUP_PER_CORE, (c + 1) * SUP_PER_CORE
        m = {
            "xe": np.ascontiguousarray(prep["xe"][s0:s1]),
            "st1": np.ascontiguousarray(prep["st1"][s0:s1]),
            "st2lo": np.ascontiguousarray(prep["st2lo"][s0:s1]),
            "st2hi": np.ascontiguousarray(prep["st2hi"][s0:s1]),
            "zidxlo": np.ascontiguousarray(prep["zidxlo"][s0:s1]),
            "zidxhi": np.ascontiguousarray(prep["zidxhi"][s0:s1]),
            "dinv2": np.ascontiguousarray(prep["dinv2"][s0:s1]),
            "w1": w1_in,
            "w2": w2_in,
        }
        if has_bias:
            m["b1b"] = np.tile(b1.astype(np.float32)[None, :], (P, 1))
            m["b2b"] = np.tile(b2.astype(np.float32)[None, :], (P, 1))
        in_maps.append(m)

    res = bass_utils.run_bass_kernel_spmd(
        nc, in_maps, core_ids=list(range(N_CORES)), trace=TRACE,
    )
    LAST_RESULTS = res

    out = np.concatenate([res.results[c]["out"] for c in range(N_CORES)], axis=0)
    return np.ascontiguousarray(out[:N_NODES]).astype(np.float32)


# revision 38
# speedup vs baseline: 1.1877x; 1.0109x over previous
"""Two-layer GCN (PyG GCNConv x2 + ReLU) on 8 Trainium2 NeuronCores via Bass.

Formulation: GCN aggregation is linear row-mixing, so for each layer
    conv(H) = A_hat @ H @ W + b      (A_hat includes self-loops, sym-norm)
and we aggregate FIRST, then matmul:
    z   = relu(A_hat @ x @ W1 + b1)
    out = A_hat @ z @ W2 + b2

Sharding: nodes split into 8 row-slabs (2560 padded rows each). Each core
aggregates + matmuls its own dst rows. The only communication is an
AllGather of z (bf16), split in two chunks so it overlaps layer-1 compute.

Aggregation: edges bucketed by 128-dst-node supertile, padded to B blocks
of 128 edges. Per block, gathered src rows G [128 edges, C] stream against
a host-built stationary selection matrix S^T [128 edges, 128 dst]:
    agg[dst, :] += S^T.T @ G       (PSUM accumulate, one matmul per block)
Row-major agg is PE-transposed into lhsT form for the @W matmul.

Layer 1 gathers are free (x is an input: the edge-ordered table xe is
host-built and streamed sequentially; self-loops included as edges).
Layer 2 gathers rows of all-gathered z via indirect DMA on GpSimd — the
hard bottleneck (~1.4us per 128-row block, POOL desc-gen serialized). To
hide it: self-loops are applied as a per-partition DVE term instead of
edges, and L2 runs in two passes (lo-half srcs right after the first AG
chunk, hi-half after the second) with lo partials staged in DRAM.
"""

import numpy as np
import ml_dtypes

N_NODES = 20000
IN_CH = 256
HID_CH = 512
N_CORES = 8
P = 128
NPAD = 20480
NSUP = NPAD // P                 # 160
SUP_PER_CORE = NSUP // N_CORES   # 20
ROWS_PER_CORE = NPAD // N_CORES  # 2560
LO_SUPS = 6                      # supertiles in the first AG chunk
LO = LO_SUPS * P                 # 768 rows per core in chunk 1
HI = ROWS_PER_CORE - LO          # 1792 rows per core in chunk 2

bf16 = ml_dtypes.bfloat16

TRACE = False
LAST_RESULTS = None

_COMPILED = {}


def _bucket(svals, dvals, wvals, nsup, cap):
    """Scatter edges (sorted by dst) into padded per-supertile slots."""
    sup = (dvals >> 7).astype(np.int64)
    cnt = np.bincount(sup, minlength=nsup)
    starts = np.zeros(nsup, np.int64)
    starts[1:] = np.cumsum(cnt)[:-1]
    slot = np.arange(len(dvals)) - starts[sup]
    gslot = sup * cap + slot
    esrc = np.zeros(nsup * cap, np.int64)
    esrc[gslot] = svals
    edstl = np.zeros(nsup * cap, np.int64)
    edstl[gslot] = dvals & 127
    ew = np.zeros(nsup * cap, np.float32)
    ew[gslot] = wvals
    return esrc, edstl, ew


def _st_blocks(edstl, ew, nsup, nblk):
    """Selection matrices in DMA layout [nsup, p, b*128+dstl]."""
    st = np.zeros((nsup * nblk * P, P), bf16)
    st[np.arange(nsup * nblk * P), edstl] = ew.astype(bf16)
    return (
        st.reshape(nsup, nblk, P, P)
        .transpose(0, 2, 1, 3)
        .reshape(nsup, P, nblk * P)
    )


def _preprocess(x, edge_index):
    src = edge_index[0].astype(np.int64)
    dst = edge_index[1].astype(np.int64)
    n = N_NODES

    deg = np.bincount(dst, minlength=n).astype(np.float32) + 1.0
    dinv = 1.0 / np.sqrt(deg)
    w = (dinv[src] * dinv[dst]).astype(np.float32)

    order = np.argsort(dst, kind="stable")
    s_src, s_dst, s_w = src[order], dst[order], w[order]

    # ---- layer 1: edges + self-loops (gathers are host-side, free) ----
    a_src = np.concatenate([s_src, np.arange(n)])
    a_dst = np.concatenate([s_dst, np.arange(n)])
    a_w = np.concatenate([s_w, dinv * dinv]).astype(np.float32)
    o1 = np.argsort(a_dst, kind="stable")
    a_src, a_dst, a_w = a_src[o1], a_dst[o1], a_w[o1]
    cnt1 = np.bincount(a_dst >> 7, minlength=NSUP)
    B1 = int(np.ceil(cnt1.max() / P))
    esrc1, edstl1, ew1 = _bucket(a_src, a_dst, a_w, NSUP, B1 * P)
    st1 = _st_blocks(edstl1, ew1, NSUP, B1)
    xbf = np.ascontiguousarray(x.astype(bf16))
    xe = (
        xbf[esrc1.reshape(NSUP, B1, P)]
        .transpose(0, 2, 1, 3)
        .reshape(NSUP, P, B1 * IN_CH)
    )

    # ---- layer 2: no self-loops; edges split by src half-slab ----
    half_pos = s_src % ROWS_PER_CORE
    lo_m = half_pos < LO
    res = {"B1": B1, "st1": st1, "xe": xe}
    for name, mask, width in (("lo", lo_m, LO), ("hi", ~lo_m, HI)):
        ms, md, mw = s_src[mask], s_dst[mask], s_w[mask]
        cnt = np.bincount(md >> 7, minlength=NSUP)
        B = int(np.ceil(cnt.max() / P))
        esrc, edstl, ew = _bucket(ms, md, mw, NSUP, B * P)
        # z_lo/z_hi row layout: [rank, width] slabs
        off = 0 if name == "lo" else LO
        zrow = (esrc // ROWS_PER_CORE) * width + (esrc % ROWS_PER_CORE) - off
        zrow = np.maximum(zrow, 0)  # padding slots (esrc=0) in hi group
        res[f"B2{name}"] = B
        res[f"st2{name}"] = _st_blocks(edstl, ew, NSUP, B)
        res[f"zidx{name}"] = (
            zrow.astype(np.int32).reshape(NSUP, B, P).transpose(0, 2, 1).copy()
        )

    dinv2 = np.zeros((NPAD,), np.float32)
    dinv2[:n] = dinv * dinv
    res["dinv2"] = dinv2.reshape(NSUP, P, 1)
    return res


def _build_program(B1, B2lo, B2hi, has_bias):
    import concourse.bass as bass
    import concourse.mybir as mybir
    import concourse.tile as tile
    from concourse.bacc import Bacc
    from concourse.masks import make_identity

    dt = mybir.dt
    nc = Bacc("TRN2", target_bir_lowering=False, debug=False, num_devices=N_CORES)

    t_xe = nc.dram_tensor("xe", [SUP_PER_CORE, P, B1 * IN_CH], dt.bfloat16,
                          kind="ExternalInput")
    t_st1 = nc.dram_tensor("st1", [SUP_PER_CORE, P, B1 * P], dt.bfloat16,
                           kind="ExternalInput")
    t_st2lo = nc.dram_tensor("st2lo", [SUP_PER_CORE, P, B2lo * P], dt.bfloat16,
                             kind="ExternalInput")
    t_st2hi = nc.dram_tensor("st2hi", [SUP_PER_CORE, P, B2hi * P], dt.bfloat16,
                             kind="ExternalInput")
    t_zidxlo = nc.dram_tensor("zidxlo", [SUP_PER_CORE, P, B2lo], dt.int32,
                              kind="ExternalInput")
    t_zidxhi = nc.dram_tensor("zidxhi", [SUP_PER_CORE, P, B2hi], dt.int32,
                              kind="ExternalInput")
    t_dinv2 = nc.dram_tensor("dinv2", [SUP_PER_CORE, P, 1], dt.float32,
                             kind="ExternalInput")
    t_w1 = nc.dram_tensor("w1", [2, P, HID_CH], dt.bfloat16, kind="ExternalInput")
    t_w2 = nc.dram_tensor("w2", [4, P, HID_CH], dt.bfloat16, kind="ExternalInput")
    if has_bias:
        t_b1 = nc.dram_tensor("b1b", [P, HID_CH], dt.float32, kind="ExternalInput")
        t_b2 = nc.dram_tensor("b2b", [P, HID_CH], dt.float32, kind="ExternalInput")
    t_out = nc.dram_tensor("out", [ROWS_PER_CORE, HID_CH], dt.float32,
                           kind="ExternalOutput")

    K1 = IN_CH // P
    K2 = HID_CH // P


    with tile.TileContext(nc) as tc:
        with (
            tc.tile_pool(name="dram", bufs=1, space="DRAM") as dram,
            tc.tile_pool(name="const", bufs=1) as cpool,
            tc.tile_pool(name="work", bufs=4) as pool,
            tc.tile_pool(name="big", bufs=3) as bigpool,
        ):
            z_slice = dram.tile([ROWS_PER_CORE, HID_CH], dt.bfloat16,
                                name="z_slice")
            z_lo = dram.tile([N_CORES * LO, HID_CH], dt.bfloat16, name="z_lo",
                             addr_space="Shared")
            z_hi = dram.tile([N_CORES * HI, HID_CH], dt.bfloat16, name="z_hi",
                             addr_space="Shared")
            partial = dram.tile([SUP_PER_CORE, P, HID_CH], dt.bfloat16,
                                name="partial")

            w1_t = cpool.tile([P, K1 * HID_CH], dt.bfloat16, name="w1_t")
            for m in range(K1):
                nc.sync.dma_start(out=w1_t[:, m * HID_CH:(m + 1) * HID_CH],
                                  in_=t_w1[m])
            w2_t = cpool.tile([P, K2 * HID_CH], dt.bfloat16, name="w2_t")
            for m in range(K2):
                nc.sync.dma_start(out=w2_t[:, m * HID_CH:(m + 1) * HID_CH],
                                  in_=t_w2[m])
            if has_bias:
                b1_t = cpool.tile([P, HID_CH], dt.float32, name="b1_t")
                nc.sync.dma_start(out=b1_t[:], in_=t_b1[:])
                b2_t = cpool.tile([P, HID_CH], dt.float32, name="b2_t")
                nc.sync.dma_start(out=b2_t[:], in_=t_b2[:])
            ident = cpool.tile([P, P], dt.float32, name="ident")
            make_identity(nc, ident[:])

            # ---------------- Layer 1 (+ chunked AllGather) ----------------
            with tc.tile_pool(name="psum1", bufs=2, space="PSUM") as psum1:
                for s in range(SUP_PER_CORE):
                    xe_t = bigpool.tile([P, B1 * IN_CH], dt.bfloat16, tag="xe",
                                        name=f"xe{s}")
                    nc.sync.dma_start(out=xe_t[:], in_=t_xe[s])
                    st_t = bigpool.tile([P, B1 * P], dt.bfloat16, tag="st1",
                                        name=f"st{s}")
                    nc.sync.dma_start(out=st_t[:], in_=t_st1[s])

                    ag1 = psum1.tile([P, IN_CH], dt.float32, tag="ag1",
                                     name=f"ag1_{s}")
                    for b in range(B1):
                        nc.tensor.matmul(
                            out=ag1[:],
                            lhsT=st_t[:, b * P:(b + 1) * P],
                            rhs=xe_t[:, b * IN_CH:(b + 1) * IN_CH],
                            start=(b == 0),
                            stop=(b == B1 - 1),
                        )
                    ag1r = pool.tile([P, IN_CH], dt.float32, tag="ag1r",
                                     name=f"ag1r{s}")
                    nc.vector.tensor_copy(out=ag1r[:], in_=ag1[:])
                    a1s = pool.tile([P, K1 * P], dt.bfloat16, tag="a1s",
                                    name=f"a1s{s}")
                    for m in range(K1):
                        tp = psum1.tile([P, P], dt.float32, tag="tp1",
                                        name=f"tp1_{s}_{m}")
                        nc.tensor.transpose(tp[:], ag1r[:, m * P:(m + 1) * P],
                                            ident[:])
                        nc.vector.tensor_copy(out=a1s[:, m * P:(m + 1) * P],
                                              in_=tp[:])
                    zp = psum1.tile([P, HID_CH], dt.float32, tag="zp",
                                    name=f"zp{s}")
                    for m in range(K1):
                        nc.tensor.matmul(
                            out=zp[:],
                            lhsT=a1s[:, m * P:(m + 1) * P],
                            rhs=w1_t[:, m * HID_CH:(m + 1) * HID_CH],
                            start=(m == 0),
                            stop=(m == K1 - 1),
                        )
                    z_t = pool.tile([P, HID_CH], dt.bfloat16, tag="z",
                                    name=f"z{s}")
                    if has_bias:
                        nc.vector.tensor_add(out=zp[:], in0=zp[:], in1=b1_t[:])
                    nc.scalar.activation(out=z_t[:], in_=zp[:],
                                         func=mybir.ActivationFunctionType.Relu)
                    nc.sync.dma_start(out=z_slice[s * P:(s + 1) * P, :],
                                      in_=z_t[:])
                    if s == LO_SUPS - 1:
                        nc.gpsimd.collective_compute(
                            "AllGather", mybir.AluOpType.bypass,
                            replica_groups=[list(range(N_CORES))],
                            ins=[z_slice[0:LO, :]],
                            outs=[z_lo.opt()],
                        )
                nc.gpsimd.collective_compute(
                    "AllGather", mybir.AluOpType.bypass,
                    replica_groups=[list(range(N_CORES))],
                    ins=[z_slice[LO:ROWS_PER_CORE, :]],
                    outs=[z_hi.opt()],
                )

            # ---------------- Layer 2 pass A: lo-half srcs ----------------
            with tc.tile_pool(name="psumA", bufs=2, space="PSUM") as psumA:
                zidxlo_all = cpool.tile([P, SUP_PER_CORE * B2lo], dt.int32,
                                        name="zidxlo_all")
                nc.sync.dma_start(
                    out=zidxlo_all[:].rearrange("p (s b) -> p s b",
                                                s=SUP_PER_CORE),
                    in_=t_zidxlo[:].rearrange("s p b -> p s b"))
                for s in range(SUP_PER_CORE):
                    g_t = bigpool.tile([P, B2lo * HID_CH], dt.bfloat16, tag="glo",
                                       name=f"glo{s}")
                    for b in range(B2lo):
                        nc.gpsimd.indirect_dma_start(
                            out=g_t[:, b * HID_CH:(b + 1) * HID_CH],
                            out_offset=None,
                            in_=z_lo[:],
                            in_offset=bass.IndirectOffsetOnAxis(
                                ap=zidxlo_all[:, s * B2lo + b:s * B2lo + b + 1],
                                axis=0),
                        )
                    st_t = bigpool.tile([P, B2lo * P], dt.bfloat16, tag="st2lo",
                                        name=f"st2lo{s}")
                    nc.sync.dma_start(out=st_t[:], in_=t_st2lo[s])
                    agA = psumA.tile([P, HID_CH], dt.float32, tag="agA",
                                     name=f"agA_{s}")
                    for b in range(B2lo):
                        nc.tensor.matmul(
                            out=agA[:],
                            lhsT=st_t[:, b * P:(b + 1) * P],
                            rhs=g_t[:, b * HID_CH:(b + 1) * HID_CH],
                            start=(b == 0),
                            stop=(b == B2lo - 1),
                        )
                    pl_t = pool.tile([P, HID_CH], dt.bfloat16, tag="pl",
                                     name=f"pl{s}")
                    nc.vector.tensor_copy(out=pl_t[:], in_=agA[:])
                    nc.sync.dma_start(out=partial[s], in_=pl_t[:])

            # ---------------- Layer 2 pass B: hi-half + finish ----------------
            with tc.tile_pool(name="psumB", bufs=2, space="PSUM") as psumB:
                zidxhi_all = cpool.tile([P, SUP_PER_CORE * B2hi], dt.int32,
                                        name="zidxhi_all")
                nc.sync.dma_start(
                    out=zidxhi_all[:].rearrange("p (s b) -> p s b",
                                                s=SUP_PER_CORE),
                    in_=t_zidxhi[:].rearrange("s p b -> p s b"))
                for s in range(SUP_PER_CORE):
                    g_t = bigpool.tile([P, B2hi * HID_CH], dt.bfloat16, tag="ghi",
                                       name=f"ghi{s}")
                    for b in range(B2hi):
                        nc.gpsimd.indirect_dma_start(
                            out=g_t[:, b * HID_CH:(b + 1) * HID_CH],
                            out_offset=None,
                            in_=z_hi[:],
                            in_offset=bass.IndirectOffsetOnAxis(
                                ap=zidxhi_all[:, s * B2hi + b:s * B2hi + b + 1],
                                axis=0),
                        )
                    st_t = bigpool.tile([P, B2hi * P], dt.bfloat16, tag="st2hi",
                                        name=f"st2hi{s}")
                    nc.sync.dma_start(out=st_t[:], in_=t_st2hi[s])
                    agB = psumB.tile([P, HID_CH], dt.float32, tag="agB",
                                     name=f"agB_{s}")
                    for b in range(B2hi):
                        nc.tensor.matmul(
                            out=agB[:],
                            lhsT=st_t[:, b * P:(b + 1) * P],
                            rhs=g_t[:, b * HID_CH:(b + 1) * HID_CH],
                            start=(b == 0),
                            stop=(b == B2hi - 1),
                        )
                    # agg2 = agB + partial_lo + dinv2 * z_own
                    pl_t = pool.tile([P, HID_CH], dt.bfloat16, tag="plb",
                                     name=f"plb{s}")
                    nc.sync.dma_start(out=pl_t[:], in_=partial[s])
                    zown_t = pool.tile([P, HID_CH], dt.bfloat16, tag="zown",
                                       name=f"zown{s}")
                    nc.sync.dma_start(out=zown_t[:],
                                      in_=z_slice[s * P:(s + 1) * P, :])
                    dinv2_t = pool.tile([P, 1], dt.float32, tag="dinv2",
                                        name=f"dinv2{s}")
                    nc.sync.dma_start(out=dinv2_t[:], in_=t_dinv2[s])
                    ag2r = pool.tile([P, HID_CH], dt.float32, tag="ag2r",
                                     name=f"ag2r{s}")
                    nc.vector.scalar_tensor_tensor(
                        out=ag2r[:], in0=zown_t[:], scalar=dinv2_t[:, :1],
                        in1=agB[:], op0=mybir.AluOpType.mult,
                        op1=mybir.AluOpType.add,
                    )
                    nc.vector.tensor_add(out=ag2r[:], in0=ag2r[:], in1=pl_t[:])

                    a2s = pool.tile([P, K2 * P], dt.bfloat16, tag="a2s",
                                    name=f"a2s{s}")
                    for m in range(K2):
                        tp = psumB.tile([P, P], dt.float32, tag="tp2",
                                        name=f"tp2_{s}_{m}")
                        nc.tensor.transpose(tp[:], ag2r[:, m * P:(m + 1) * P],
                                            ident[:])
                        nc.vector.tensor_copy(out=a2s[:, m * P:(m + 1) * P],
                                              in_=tp[:])
                    op = psumB.tile([P, HID_CH], dt.float32, tag="op",
                                    name=f"op{s}")
                    for m in range(K2):
                        nc.tensor.matmul(
                            out=op[:],
                            lhsT=a2s[:, m * P:(m + 1) * P],
                            rhs=w2_t[:, m * HID_CH:(m + 1) * HID_CH],
                            start=(m == 0),
                            stop=(m == K2 - 1),
                        )
                    o_t = pool.tile([P, HID_CH], dt.float32, tag="o",
                                    name=f"o{s}")
                    if has_bias:
                        nc.vector.tensor_add(out=o_t[:], in0=op[:], in1=b2_t[:])
                    else:
                        nc.vector.tensor_copy(out=o_t[:], in_=op[:])
                    nc.sync.dma_start(out=t_out[s * P:(s + 1) * P, :], in_=o_t[:])

    nc.compile()
    return nc


def kernel(x, edge_index, W1, b1, W2, b2):
    global LAST_RESULTS
    from concourse import bass_utils

    x = np.asarray(x, np.float32)
    edge_index = np.asarray(edge_index)
    W1 = np.asarray(W1, np.float32)
    b1 = np.asarray(b1, np.float32)
    W2 = np.asarray(W2, np.float32)
    b2 = np.asarray(b2, np.float32)

    prep = _preprocess(x, edge_index)
    B1, B2lo, B2hi = prep["B1"], prep["B2lo"], prep["B2hi"]
    has_bias = bool(np.any(b1) or np.any(b2))

    key = (B1, B2lo, B2hi, has_bias)
    if key not in _COMPILED:
        _COMPILED[key] = _build_program(B1, B2lo, B2hi, has_bias)
    nc = _COMPILED[key]

    w1_in = np.ascontiguousarray(W1.astype(bf16).reshape(2, P, HID_CH))
    w2_in = np.ascontiguousarray(W2.astype(bf16).reshape(4, P, HID_CH))

    in_maps = []
    for c in range(N_CORES):
        s0, s1 = c * SUP_PER_CORE, (c + 1) * SUP_PER_CORE
        m = {
            "xe": np.ascontiguousarray(prep["xe"][s0:s1]),
            "st1": np.ascontiguousarray(prep["st1"][s0:s1]),
            "st2lo": np.ascontiguousarray(prep["st2lo"][s0:s1]),
            "st2hi": np.ascontiguousarray(prep["st2hi"][s0:s1]),
            "zidxlo": np.ascontiguousarray(prep["zidxlo"][s0:s1]),
            "zidxhi": np.ascontiguousarray(prep["zidxhi"][s0:s1]),
            "dinv2": np.ascontiguousarray(prep["dinv2"][s0:s1]),
            "w1": w1_in,
            "w2": w2_in,
        }
        if has_bias:
            m["b1b"] = np.tile(b1.astype(np.float32)[None, :], (P, 1))
            m["b2b"] = np.tile(b2.astype(np.float32)[None, :], (P, 1))
        in_maps.append(m)

    res = bass_utils.run_bass_kernel_spmd(
        nc, in_maps, core_ids=list(range(N_CORES)), trace=TRACE,
    )
    LAST_RESULTS = res

    out = np.concatenate([res.results[c]["out"] for c in range(N_CORES)], axis=0)
    return np.ascontiguousarray(out[:N_NODES]).astype(np.float32)


# revision 39
# speedup vs baseline: 1.1882x; 1.0004x over previous
"""Two-layer GCN (PyG GCNConv x2 + ReLU) on 8 Trainium2 NeuronCores via Bass.

Formulation: GCN aggregation is linear row-mixing, so for each layer
    conv(H) = A_hat @ H @ W + b      (A_hat includes self-loops, sym-norm)
and we aggregate FIRST, then matmul:
    z   = relu(A_hat @ x @ W1 + b1)
    out = A_hat @ z @ W2 + b2

Sharding: nodes split into 8 row-slabs (2560 padded rows each). Each core
aggregates + matmuls its own dst rows. The only communication is an
AllGather of z (bf16), split in two chunks so it overlaps layer-1 compute.

Aggregation: edges bucketed by 128-dst-node supertile, padded to B blocks
of 128 edges. Per block, gathered src rows G [128 edges, C] stream against
a host-built stationary selection matrix S^T [128 edges, 128 dst]:
    agg[dst, :] += S^T.T @ G       (PSUM accumulate, one matmul per block)
Row-major agg is PE-transposed into lhsT form for the @W matmul.

Layer 1 gathers are free (x is an input: the edge-ordered table xe is
host-built and streamed sequentially; self-loops included as edges).
Layer 2 gathers rows of all-gathered z via indirect DMA on GpSimd — the
hard bottleneck (~1.4us per 128-row block, POOL desc-gen serialized). To
hide it: self-loops are applied as a per-partition DVE term instead of
edges, and L2 runs in two passes (lo-half srcs right after the first AG
chunk, hi-half after the second) with lo partials staged in DRAM.
"""

import numpy as np
import ml_dtypes

N_NODES = 20000
IN_CH = 256
HID_CH = 512
N_CORES = 8
P = 128
NPAD = 20480
NSUP = NPAD // P                 # 160
SUP_PER_CORE = NSUP // N_CORES   # 20
ROWS_PER_CORE = NPAD // N_CORES  # 2560
LO_SUPS = 6                      # supertiles in the first AG chunk
LO = LO_SUPS * P                 # 768 rows per core in chunk 1
HI = ROWS_PER_CORE - LO          # 1792 rows per core in chunk 2

bf16 = ml_dtypes.bfloat16

TRACE = False
LAST_RESULTS = None

_COMPILED = {}


def _bucket(svals, dvals, wvals, nsup, cap):
    """Scatter edges (sorted by dst) into padded per-supertile slots."""
    sup = (dvals >> 7).astype(np.int64)
    cnt = np.bincount(sup, minlength=nsup)
    starts = np.zeros(nsup, np.int64)
    starts[1:] = np.cumsum(cnt)[:-1]
    slot = np.arange(len(dvals)) - starts[sup]
    gslot = sup * cap + slot
    esrc = np.zeros(nsup * cap, np.int64)
    esrc[gslot] = svals
    edstl = np.zeros(nsup * cap, np.int64)
    edstl[gslot] = dvals & 127
    ew = np.zeros(nsup * cap, np.float32)
    ew[gslot] = wvals
    return esrc, edstl, ew


def _st_blocks(edstl, ew, nsup, nblk):
    """Selection matrices in DMA layout [nsup, p, b*128+dstl]."""
    st = np.zeros((nsup * nblk * P, P), bf16)
    st[np.arange(nsup * nblk * P), edstl] = ew.astype(bf16)
    return (
        st.reshape(nsup, nblk, P, P)
        .transpose(0, 2, 1, 3)
        .reshape(nsup, P, nblk * P)
    )


def _preprocess(x, edge_index):
    src = edge_index[0].astype(np.int64)
    dst = edge_index[1].astype(np.int64)
    n = N_NODES

    deg = np.bincount(dst, minlength=n).astype(np.float32) + 1.0
    dinv = 1.0 / np.sqrt(deg)
    w = (dinv[src] * dinv[dst]).astype(np.float32)

    order = np.argsort(dst, kind="stable")
    s_src, s_dst, s_w = src[order], dst[order], w[order]

    # ---- layer 1: edges + self-loops (gathers are host-side, free) ----
    a_src = np.concatenate([s_src, np.arange(n)])
    a_dst = np.concatenate([s_dst, np.arange(n)])
    a_w = np.concatenate([s_w, dinv * dinv]).astype(np.float32)
    o1 = np.argsort(a_dst, kind="stable")
    a_src, a_dst, a_w = a_src[o1], a_dst[o1], a_w[o1]
    cnt1 = np.bincount(a_dst >> 7, minlength=NSUP)
    B1 = int(np.ceil(cnt1.max() / P))
    esrc1, edstl1, ew1 = _bucket(a_src, a_dst, a_w, NSUP, B1 * P)
    st1 = _st_blocks(edstl1, ew1, NSUP, B1)
    xbf = np.ascontiguousarray(x.astype(bf16))
    xe = (
        xbf[esrc1.reshape(NSUP, B1, P)]
        .transpose(0, 2, 1, 3)
        .reshape(NSUP, P, B1 * IN_CH)
    )

    # ---- layer 2: no self-loops; edges split by src half-slab ----
    half_pos = s_src % ROWS_PER_CORE
    lo_m = half_pos < LO
    res = {"B1": B1, "st1": st1, "xe": xe}
    for name, mask, width in (("lo", lo_m, LO), ("hi", ~lo_m, HI)):
        ms, md, mw = s_src[mask], s_dst[mask], s_w[mask]
        cnt = np.bincount(md >> 7, minlength=NSUP)
        B = int(np.ceil(cnt.max() / P))
        esrc, edstl, ew = _bucket(ms, md, mw, NSUP, B * P)
        # z_lo/z_hi row layout: [rank, width] slabs
        off = 0 if name == "lo" else LO
        zrow = (esrc // ROWS_PER_CORE) * width + (esrc % ROWS_PER_CORE) - off
        zrow = np.maximum(zrow, 0)  # padding slots (esrc=0) in hi group
        res[f"B2{name}"] = B
        res[f"st2{name}"] = _st_blocks(edstl, ew, NSUP, B)
        res[f"zidx{name}"] = (
            zrow.astype(np.int32).reshape(NSUP, B, P).transpose(0, 2, 1).copy()
        )

    dinv2 = np.zeros((NPAD,), np.float32)
    dinv2[:n] = dinv * dinv
    res["dinv2"] = dinv2.reshape(NSUP, P, 1)
    return res


def _build_program(B1, B2lo, B2hi, has_bias):
    import concourse.bass as bass
    import concourse.mybir as mybir
    import concourse.tile as tile
    from concourse.bacc import Bacc
    from concourse.masks import make_identity

    dt = mybir.dt
    nc = Bacc("TRN2", target_bir_lowering=False, debug=False, num_devices=N_CORES)

    t_xe = nc.dram_tensor("xe", [SUP_PER_CORE, P, B1 * IN_CH], dt.bfloat16,
                          kind="ExternalInput")
    t_st1 = nc.dram_tensor("st1", [SUP_PER_CORE, P, B1 * P], dt.bfloat16,
                           kind="ExternalInput")
    t_st2lo = nc.dram_tensor("st2lo", [SUP_PER_CORE, P, B2lo * P], dt.bfloat16,
                             kind="ExternalInput")
    t_st2hi = nc.dram_tensor("st2hi", [SUP_PER_CORE, P, B2hi * P], dt.bfloat16,
                             kind="ExternalInput")
    t_zidxlo = nc.dram_tensor("zidxlo", [SUP_PER_CORE, P, B2lo], dt.int32,
                              kind="ExternalInput")
    t_zidxhi = nc.dram_tensor("zidxhi", [SUP_PER_CORE, P, B2hi], dt.int32,
                              kind="ExternalInput")
    t_dinv2 = nc.dram_tensor("dinv2", [SUP_PER_CORE, P, 1], dt.float32,
                             kind="ExternalInput")
    t_w1 = nc.dram_tensor("w1", [2, P, HID_CH], dt.bfloat16, kind="ExternalInput")
    t_w2 = nc.dram_tensor("w2", [4, P, HID_CH], dt.bfloat16, kind="ExternalInput")
    if has_bias:
        t_b1 = nc.dram_tensor("b1b", [P, HID_CH], dt.float32, kind="ExternalInput")
        t_b2 = nc.dram_tensor("b2b", [P, HID_CH], dt.float32, kind="ExternalInput")
    t_out = nc.dram_tensor("out", [ROWS_PER_CORE, HID_CH], dt.float32,
                           kind="ExternalOutput")

    K1 = IN_CH // P
    K2 = HID_CH // P


    with tile.TileContext(nc) as tc:
        with (
            tc.tile_pool(name="dram", bufs=1, space="DRAM") as dram,
            tc.tile_pool(name="const", bufs=1) as cpool,
            tc.tile_pool(name="work", bufs=4) as pool,
            tc.tile_pool(name="big", bufs=3) as bigpool,
            tc.tile_pool(name="gbuf", bufs=5) as gpool,
        ):
            z_slice = dram.tile([ROWS_PER_CORE, HID_CH], dt.bfloat16,
                                name="z_slice")
            z_lo = dram.tile([N_CORES * LO, HID_CH], dt.bfloat16, name="z_lo",
                             addr_space="Shared")
            z_hi = dram.tile([N_CORES * HI, HID_CH], dt.bfloat16, name="z_hi",
                             addr_space="Shared")
            partial = dram.tile([SUP_PER_CORE, P, HID_CH], dt.bfloat16,
                                name="partial")

            w1_t = cpool.tile([P, K1 * HID_CH], dt.bfloat16, name="w1_t")
            for m in range(K1):
                nc.sync.dma_start(out=w1_t[:, m * HID_CH:(m + 1) * HID_CH],
                                  in_=t_w1[m])
            w2_t = cpool.tile([P, K2 * HID_CH], dt.bfloat16, name="w2_t")
            for m in range(K2):
                nc.sync.dma_start(out=w2_t[:, m * HID_CH:(m + 1) * HID_CH],
                                  in_=t_w2[m])
            if has_bias:
                b1_t = cpool.tile([P, HID_CH], dt.float32, name="b1_t")
                nc.sync.dma_start(out=b1_t[:], in_=t_b1[:])
                b2_t = cpool.tile([P, HID_CH], dt.float32, name="b2_t")
                nc.sync.dma_start(out=b2_t[:], in_=t_b2[:])
            ident = cpool.tile([P, P], dt.float32, name="ident")
            make_identity(nc, ident[:])

            # ---------------- Layer 1 (+ chunked AllGather) ----------------
            with tc.tile_pool(name="psum1", bufs=2, space="PSUM") as psum1:
                for s in range(SUP_PER_CORE):
                    xe_t = bigpool.tile([P, B1 * IN_CH], dt.bfloat16, tag="xe",
                                        name=f"xe{s}")
                    nc.sync.dma_start(out=xe_t[:], in_=t_xe[s])
                    st_t = bigpool.tile([P, B1 * P], dt.bfloat16, tag="st1",
                                        name=f"st{s}")
                    nc.sync.dma_start(out=st_t[:], in_=t_st1[s])

                    ag1 = psum1.tile([P, IN_CH], dt.float32, tag="ag1",
                                     name=f"ag1_{s}")
                    for b in range(B1):
                        nc.tensor.matmul(
                            out=ag1[:],
                            lhsT=st_t[:, b * P:(b + 1) * P],
                            rhs=xe_t[:, b * IN_CH:(b + 1) * IN_CH],
                            start=(b == 0),
                            stop=(b == B1 - 1),
                        )
                    ag1r = pool.tile([P, IN_CH], dt.float32, tag="ag1r",
                                     name=f"ag1r{s}")
                    nc.vector.tensor_copy(out=ag1r[:], in_=ag1[:])
                    a1s = pool.tile([P, K1 * P], dt.bfloat16, tag="a1s",
                                    name=f"a1s{s}")
                    for m in range(K1):
                        tp = psum1.tile([P, P], dt.float32, tag="tp1",
                                        name=f"tp1_{s}_{m}")
                        nc.tensor.transpose(tp[:], ag1r[:, m * P:(m + 1) * P],
                                            ident[:])
                        nc.vector.tensor_copy(out=a1s[:, m * P:(m + 1) * P],
                                              in_=tp[:])
                    zp = psum1.tile([P, HID_CH], dt.float32, tag="zp",
                                    name=f"zp{s}")
                    for m in range(K1):
                        nc.tensor.matmul(
                            out=zp[:],
                            lhsT=a1s[:, m * P:(m + 1) * P],
                            rhs=w1_t[:, m * HID_CH:(m + 1) * HID_CH],
                            start=(m == 0),
                            stop=(m == K1 - 1),
                        )
                    z_t = pool.tile([P, HID_CH], dt.bfloat16, tag="z",
                                    name=f"z{s}")
                    if has_bias:
                        nc.vector.tensor_add(out=zp[:], in0=zp[:], in1=b1_t[:])
                    nc.scalar.activation(out=z_t[:], in_=zp[:],
                                         func=mybir.ActivationFunctionType.Relu)
                    nc.sync.dma_start(out=z_slice[s * P:(s + 1) * P, :],
                                      in_=z_t[:])
                    if s == LO_SUPS - 1:
                        nc.gpsimd.collective_compute(
                            "AllGather", mybir.AluOpType.bypass,
                            replica_groups=[list(range(N_CORES))],
                            ins=[z_slice[0:LO, :]],
                            outs=[z_lo.opt()],
                        )
                nc.gpsimd.collective_compute(
                    "AllGather", mybir.AluOpType.bypass,
                    replica_groups=[list(range(N_CORES))],
                    ins=[z_slice[LO:ROWS_PER_CORE, :]],
                    outs=[z_hi.opt()],
                )

            # ---------------- Layer 2 pass A: lo-half srcs ----------------
            with tc.tile_pool(name="psumA", bufs=2, space="PSUM") as psumA:
                zidxlo_all = cpool.tile([P, SUP_PER_CORE * B2lo], dt.int32,
                                        name="zidxlo_all")
                nc.sync.dma_start(
                    out=zidxlo_all[:].rearrange("p (s b) -> p s b",
                                                s=SUP_PER_CORE),
                    in_=t_zidxlo[:].rearrange("s p b -> p s b"))
                for s in range(SUP_PER_CORE):
                    g_t = gpool.tile([P, B2lo * HID_CH], dt.bfloat16, tag="glo",
                                     name=f"glo{s}")
                    for b in range(B2lo):
                        nc.gpsimd.indirect_dma_start(
                            out=g_t[:, b * HID_CH:(b + 1) * HID_CH],
                            out_offset=None,
                            in_=z_lo[:],
                            in_offset=bass.IndirectOffsetOnAxis(
                                ap=zidxlo_all[:, s * B2lo + b:s * B2lo + b + 1],
                                axis=0),
                        )
                    st_t = bigpool.tile([P, B2lo * P], dt.bfloat16, tag="st2lo",
                                        name=f"st2lo{s}")
                    nc.sync.dma_start(out=st_t[:], in_=t_st2lo[s])
                    agA = psumA.tile([P, HID_CH], dt.float32, tag="agA",
                                     name=f"agA_{s}")
                    for b in range(B2lo):
                        nc.tensor.matmul(
                            out=agA[:],
                            lhsT=st_t[:, b * P:(b + 1) * P],
                            rhs=g_t[:, b * HID_CH:(b + 1) * HID_CH],
                            start=(b == 0),
                            stop=(b == B2lo - 1),
                        )
                    pl_t = pool.tile([P, HID_CH], dt.bfloat16, tag="pl",
                                     name=f"pl{s}")
                    nc.vector.tensor_copy(out=pl_t[:], in_=agA[:])
                    nc.sync.dma_start(out=partial[s], in_=pl_t[:])

            # ---------------- Layer 2 pass B: hi-half + finish ----------------
            with tc.tile_pool(name="psumB", bufs=2, space="PSUM") as psumB:
                zidxhi_all = cpool.tile([P, SUP_PER_CORE * B2hi], dt.int32,
                                        name="zidxhi_all")
                nc.sync.dma_start(
                    out=zidxhi_all[:].rearrange("p (s b) -> p s b",
                                                s=SUP_PER_CORE),
                    in_=t_zidxhi[:].rearrange("s p b -> p s b"))
                for s in range(SUP_PER_CORE):
                    g_t = gpool.tile([P, B2hi * HID_CH], dt.bfloat16, tag="ghi",
                                     name=f"ghi{s}")
                    for b in range(B2hi):
                        nc.gpsimd.indirect_dma_start(
                            out=g_t[:, b * HID_CH:(b + 1) * HID_CH],
                            out_offset=None,
                            in_=z_hi[:],
                            in_offset=bass.IndirectOffsetOnAxis(
                                ap=zidxhi_all[:, s * B2hi + b:s * B2hi + b + 1],
                                axis=0),
                        )
                    st_t = bigpool.tile([P, B2hi * P], dt.bfloat16, tag="st2hi",
                                        name=f"st2hi{s}")
                    nc.sync.dma_start(out=st_t[:], in_=t_st2hi[s])
                    agB = psumB.tile([P, HID_CH], dt.float32, tag="agB",
                                     name=f"agB_{s}")
                    for b in range(B2hi):
                        nc.tensor.matmul(
                            out=agB[:],
                            lhsT=st_t[:, b * P:(b + 1) * P],
                            rhs=g_t[:, b * HID_CH:(b + 1) * HID_CH],
                            start=(b == 0),
                            stop=(b == B2hi - 1),
                        )
                    # agg2 = agB + partial_lo + dinv2 * z_own
                    pl_t = pool.tile([P, HID_CH], dt.bfloat16, tag="plb",
                                     name=f"plb{s}")
                    nc.sync.dma_start(out=pl_t[:], in_=partial[s])
                    zown_t = pool.tile([P, HID_CH], dt.bfloat16, tag="zown",
                                       name=f"zown{s}")
                    nc.sync.dma_start(out=zown_t[:],
                                      in_=z_slice[s * P:(s + 1) * P, :])
                    dinv2_t = pool.tile([P, 1], dt.float32, tag="dinv2",
                                        name=f"dinv2{s}")
                    nc.sync.dma_start(out=dinv2_t[:], in_=t_dinv2[s])
                    ag2r = pool.tile([P, HID_CH], dt.float32, tag="ag2r",
                                     name=f"ag2r{s}")
                    nc.vector.scalar_tensor_tensor(
                        out=ag2r[:], in0=zown_t[:], scalar=dinv2_t[:, :1],
                        in1=agB[:], op0=mybir.AluOpType.mult,
                        op1=mybir.AluOpType.add,
                    )
                    nc.vector.tensor_add(out=ag2r[:], in0=ag2r[:], in1=pl_t[:])

                    a2s = pool.tile([P, K2 * P], dt.bfloat16, tag="a2s",
                                    name=f"a2s{s}")
                    for m in range(K2):
                        tp = psumB.tile([P, P], dt.float32, tag="tp2",
                                        name=f"tp2_{s}_{m}")
                        nc.tensor.transpose(tp[:], ag2r[:, m * P:(m + 1) * P],
                                            ident[:])
                        nc.vector.tensor_copy(out=a2s[:, m * P:(m + 1) * P],
                                              in_=tp[:])
                    op = psumB.tile([P, HID_CH], dt.float32, tag="op",
                                    name=f"op{s}")
                    for m in range(K2):
                        nc.tensor.matmul(
                            out=op[:],
                            lhsT=a2s[:, m * P:(m + 1) * P],
                            rhs=w2_t[:, m * HID_CH:(m + 1) * HID_CH],
                            start=(m == 0),
                            stop=(m == K2 - 1),
                        )
                    o_t = pool.tile([P, HID_CH], dt.float32, tag="o",
                                    name=f"o{s}")
                    if has_bias:
                        nc.vector.tensor_add(out=o_t[:], in0=op[:], in1=b2_t[:])
                    else:
                        nc.vector.tensor_copy(out=o_t[:], in_=op[:])
                    nc.sync.dma_start(out=t_out[s * P:(s + 1) * P, :], in_=o_t[:])

    nc.compile()
    return nc


def kernel(x, edge_index, W1, b1, W2, b2):
    global LAST_RESULTS
    from concourse import bass_utils

    x = np.asarray(x, np.float32)
    edge_index = np.asarray(edge_index)
    W1 = np.asarray(W1, np.float32)
    b1 = np.asarray(b1, np.float32)
    W2 = np.asarray(W2, np.float32)
    b2 = np.asarray(b2, np.float32)

    prep = _preprocess(x, edge_index)
    B1, B2lo, B2hi = prep["B1"], prep["B2lo"], prep["B2hi"]
    has_bias = bool(np.any(b1) or np.any(b2))

    key = (B1, B2lo, B2hi, has_bias)
    if key not in _COMPILED:
        _COMPILED[key] = _build_program(B1, B2lo, B2hi, has_bias)
    nc = _COMPILED[key]

    w1_in = np.ascontiguousarray(W1.astype(bf16).reshape(2, P, HID_CH))
    w2_in = np.ascontiguousarray(W2.astype(bf16).reshape(4, P, HID_CH))

    in_maps = []
    for c in range(N_CORES):
        s0, s1 = c * SUP_PER_CORE, (c + 1) * SUP_PER_CORE
        m = {
            "xe": np.ascontiguousarray(prep["xe"][s0:s1]),
            "st1": np.ascontiguousarray(prep["st1"][s0:s1]),
            "st2lo": np.ascontiguousarray(prep["st2lo"][s0:s1]),
            "st2hi": np.ascontiguousarray(prep["st2hi"][s0:s1]),
            "zidxlo": np.ascontiguousarray(prep["zidxlo"][s0:s1]),
            "zidxhi": np.ascontiguousarray(prep["zidxhi"][s0:s1]),
            "dinv2": np.ascontiguousarray(prep["dinv2"][s0:s1]),
            "w1": w1_in,
            "w2": w2_in,
        }
        if has_bias:
            m["b1b"] = np.tile(b1.astype(np.float32)[None, :], (P, 1))
            m["b2b"] = np.tile(b2.astype(np.float32)[None, :], (P, 1))
        in_maps.append(m)

    res = bass_utils.run_bass_kernel_spmd(
        nc, in_maps, core_ids=list(range(N_CORES)), trace=TRACE,
    )
    LAST_RESULTS = res

    out = np.concatenate([res.results[c]["out"] for c in range(N_CORES)], axis=0)
    return np.ascontiguousarray(out[:N_NODES]).astype(np.float32)


# revision 41
# speedup vs baseline: 1.1991x; 1.0092x over previous
"""Two-layer GCN (PyG GCNConv x2 + ReLU) on 8 Trainium2 NeuronCores via Bass.

Formulation: GCN aggregation is linear row-mixing, so for each layer
    conv(H) = A_hat @ H @ W + b      (A_hat includes self-loops, sym-norm)
and we aggregate FIRST, then matmul:
    z   = relu(A_hat @ x @ W1 + b1)
    out = A_hat @ z @ W2 + b2

Sharding: nodes split into 8 row-slabs (2560 padded rows each). Each core
aggregates + matmuls its own dst rows. The only communication is an
AllGather of z (bf16), split in two chunks so it overlaps layer-1 compute.

Aggregation: edges bucketed by 128-dst-node supertile, padded to B blocks
of 128 edges. Per block, gathered src rows G [128 edges, C] stream against
a host-built stationary selection matrix S^T [128 edges, 128 dst]:
    agg[dst, :] += S^T.T @ G       (PSUM accumulate, one matmul per block)
Row-major agg is PE-transposed into lhsT form for the @W matmul.

Layer 1 gathers are free (x is an input: the edge-ordered table xe is
host-built and streamed sequentially; self-loops included as edges).
Layer 2 gathers rows of all-gathered z via indirect DMA on GpSimd — the
hard bottleneck (~1.4us per 128-row block, POOL desc-gen serialized). To
hide it: self-loops are applied as a per-partition DVE term instead of
edges, and L2 runs in two passes (lo-half srcs right after the first AG
chunk, hi-half after the second) with lo partials staged in DRAM.
"""

import numpy as np
import ml_dtypes

N_NODES = 20000
IN_CH = 256
HID_CH = 512
N_CORES = 8
P = 128
NPAD = 20480
NSUP = NPAD // P                 # 160
SUP_PER_CORE = NSUP // N_CORES   # 20
ROWS_PER_CORE = NPAD // N_CORES  # 2560
LO_SUPS = 6                      # supertiles in the first AG chunk
LO = LO_SUPS * P                 # 768 rows per core in chunk 1
HI = ROWS_PER_CORE - LO          # 1792 rows per core in chunk 2

bf16 = ml_dtypes.bfloat16

TRACE = False
LAST_RESULTS = None

_COMPILED = {}


def _bucket(svals, dvals, wvals, nsup, cap):
    """Scatter edges (sorted by dst) into padded per-supertile slots."""
    sup = (dvals >> 7).astype(np.int64)
    cnt = np.bincount(sup, minlength=nsup)
    starts = np.zeros(nsup, np.int64)
    starts[1:] = np.cumsum(cnt)[:-1]
    slot = np.arange(len(dvals)) - starts[sup]
    gslot = sup * cap + slot
    esrc = np.zeros(nsup * cap, np.int64)
    esrc[gslot] = svals
    edstl = np.zeros(nsup * cap, np.int64)
    edstl[gslot] = dvals & 127
    ew = np.zeros(nsup * cap, np.float32)
    ew[gslot] = wvals
    return esrc, edstl, ew


def _st_blocks(edstl, ew, nsup, nblk):
    """Selection matrices in DMA layout [nsup, p, b*128+dstl]."""
    st = np.zeros((nsup * nblk * P, P), bf16)
    st[np.arange(nsup * nblk * P), edstl] = ew.astype(bf16)
    return (
        st.reshape(nsup, nblk, P, P)
        .transpose(0, 2, 1, 3)
        .reshape(nsup, P, nblk * P)
    )


def _preprocess(x, edge_index):
    src = edge_index[0].astype(np.int64)
    dst = edge_index[1].astype(np.int64)
    n = N_NODES

    deg = np.bincount(dst, minlength=n).astype(np.float32) + 1.0
    dinv = 1.0 / np.sqrt(deg)
    w = (dinv[src] * dinv[dst]).astype(np.float32)

    order = np.argsort(dst, kind="stable")
    s_src, s_dst, s_w = src[order], dst[order], w[order]

    # ---- layer 1: edges + self-loops (gathers are host-side, free) ----
    a_src = np.concatenate([s_src, np.arange(n)])
    a_dst = np.concatenate([s_dst, np.arange(n)])
    a_w = np.concatenate([s_w, dinv * dinv]).astype(np.float32)
    o1 = np.argsort(a_dst, kind="stable")
    a_src, a_dst, a_w = a_src[o1], a_dst[o1], a_w[o1]
    cnt1 = np.bincount(a_dst >> 7, minlength=NSUP)
    B1 = int(np.ceil(cnt1.max() / P))
    esrc1, edstl1, ew1 = _bucket(a_src, a_dst, a_w, NSUP, B1 * P)
    st1 = _st_blocks(edstl1, ew1, NSUP, B1)
    xbf = np.ascontiguousarray(x.astype(bf16))
    xe = (
        xbf[esrc1.reshape(NSUP, B1, P)]
        .transpose(0, 2, 1, 3)
        .reshape(NSUP, P, B1 * IN_CH)
    )

    # ---- layer 2: no self-loops; edges split by src half-slab ----
    half_pos = s_src % ROWS_PER_CORE
    lo_m = half_pos < LO
    res = {"B1": B1, "st1": st1, "xe": xe}
    for name, mask, width in (("lo", lo_m, LO), ("hi", ~lo_m, HI)):
        ms, md, mw = s_src[mask], s_dst[mask], s_w[mask]
        cnt = np.bincount(md >> 7, minlength=NSUP)
        B = int(np.ceil(cnt.max() / P))
        esrc, edstl, ew = _bucket(ms, md, mw, NSUP, B * P)
        # z_lo/z_hi row layout: [rank, width] slabs
        off = 0 if name == "lo" else LO
        zrow = (esrc // ROWS_PER_CORE) * width + (esrc % ROWS_PER_CORE) - off
        zrow = np.maximum(zrow, 0)
        # padding slots get an out-of-bounds index so the DGE skips them
        zrow[ew == 0.0] = 1 << 20
        res[f"B2{name}"] = B
        res[f"st2{name}"] = _st_blocks(edstl, ew, NSUP, B)
        res[f"zidx{name}"] = (
            zrow.astype(np.int32).reshape(NSUP, B, P).transpose(0, 2, 1).copy()
        )

    dinv2 = np.zeros((NPAD,), np.float32)
    dinv2[:n] = dinv * dinv
    res["dinv2"] = dinv2.reshape(NSUP, P, 1)
    return res


def _build_program(B1, B2lo, B2hi, has_bias):
    import concourse.bass as bass
    import concourse.mybir as mybir
    import concourse.tile as tile
    from concourse.bacc import Bacc
    from concourse.masks import make_identity

    dt = mybir.dt
    nc = Bacc("TRN2", target_bir_lowering=False, debug=False, num_devices=N_CORES)

    t_xe = nc.dram_tensor("xe", [SUP_PER_CORE, P, B1 * IN_CH], dt.bfloat16,
                          kind="ExternalInput")
    t_st1 = nc.dram_tensor("st1", [SUP_PER_CORE, P, B1 * P], dt.bfloat16,
                           kind="ExternalInput")
    t_st2lo = nc.dram_tensor("st2lo", [SUP_PER_CORE, P, B2lo * P], dt.bfloat16,
                             kind="ExternalInput")
    t_st2hi = nc.dram_tensor("st2hi", [SUP_PER_CORE, P, B2hi * P], dt.bfloat16,
                             kind="ExternalInput")
    t_zidxlo = nc.dram_tensor("zidxlo", [SUP_PER_CORE, P, B2lo], dt.int32,
                              kind="ExternalInput")
    t_zidxhi = nc.dram_tensor("zidxhi", [SUP_PER_CORE, P, B2hi], dt.int32,
                              kind="ExternalInput")
    t_dinv2 = nc.dram_tensor("dinv2", [SUP_PER_CORE, P, 1], dt.float32,
                             kind="ExternalInput")
    t_w1 = nc.dram_tensor("w1", [2, P, HID_CH], dt.bfloat16, kind="ExternalInput")
    t_w2 = nc.dram_tensor("w2", [4, P, HID_CH], dt.bfloat16, kind="ExternalInput")
    if has_bias:
        t_b1 = nc.dram_tensor("b1b", [P, HID_CH], dt.float32, kind="ExternalInput")
        t_b2 = nc.dram_tensor("b2b", [P, HID_CH], dt.float32, kind="ExternalInput")
    t_out = nc.dram_tensor("out", [ROWS_PER_CORE, HID_CH], dt.float32,
                           kind="ExternalOutput")

    K1 = IN_CH // P
    K2 = HID_CH // P


    with tile.TileContext(nc) as tc:
        with (
            tc.tile_pool(name="dram", bufs=1, space="DRAM") as dram,
            tc.tile_pool(name="const", bufs=1) as cpool,
            tc.tile_pool(name="work", bufs=4) as pool,
            tc.tile_pool(name="big", bufs=3) as bigpool,
            tc.tile_pool(name="gbuf", bufs=5) as gpool,
        ):
            z_slice = dram.tile([ROWS_PER_CORE, HID_CH], dt.bfloat16,
                                name="z_slice")
            z_lo = dram.tile([N_CORES * LO, HID_CH], dt.bfloat16, name="z_lo",
                             addr_space="Shared")
            z_hi = dram.tile([N_CORES * HI, HID_CH], dt.bfloat16, name="z_hi",
                             addr_space="Shared")
            partial = dram.tile([SUP_PER_CORE, P, HID_CH], dt.bfloat16,
                                name="partial")

            w1_t = cpool.tile([P, K1 * HID_CH], dt.bfloat16, name="w1_t")
            for m in range(K1):
                nc.sync.dma_start(out=w1_t[:, m * HID_CH:(m + 1) * HID_CH],
                                  in_=t_w1[m])
            w2_t = cpool.tile([P, K2 * HID_CH], dt.bfloat16, name="w2_t")
            for m in range(K2):
                nc.sync.dma_start(out=w2_t[:, m * HID_CH:(m + 1) * HID_CH],
                                  in_=t_w2[m])
            if has_bias:
                b1_t = cpool.tile([P, HID_CH], dt.float32, name="b1_t")
                nc.sync.dma_start(out=b1_t[:], in_=t_b1[:])
                b2_t = cpool.tile([P, HID_CH], dt.float32, name="b2_t")
                nc.sync.dma_start(out=b2_t[:], in_=t_b2[:])
            ident = cpool.tile([P, P], dt.float32, name="ident")
            make_identity(nc, ident[:])

            # ---------------- Layer 1 (+ chunked AllGather) ----------------
            with tc.tile_pool(name="psum1", bufs=2, space="PSUM") as psum1:
                for s in range(SUP_PER_CORE):
                    xe_t = bigpool.tile([P, B1 * IN_CH], dt.bfloat16, tag="xe",
                                        name=f"xe{s}")
                    nc.sync.dma_start(out=xe_t[:], in_=t_xe[s])
                    st_t = bigpool.tile([P, B1 * P], dt.bfloat16, tag="st1",
                                        name=f"st{s}")
                    nc.sync.dma_start(out=st_t[:], in_=t_st1[s])

                    ag1 = psum1.tile([P, IN_CH], dt.float32, tag="ag1",
                                     name=f"ag1_{s}")
                    for b in range(B1):
                        nc.tensor.matmul(
                            out=ag1[:],
                            lhsT=st_t[:, b * P:(b + 1) * P],
                            rhs=xe_t[:, b * IN_CH:(b + 1) * IN_CH],
                            start=(b == 0),
                            stop=(b == B1 - 1),
                        )
                    ag1r = pool.tile([P, IN_CH], dt.float32, tag="ag1r",
                                     name=f"ag1r{s}")
                    nc.vector.tensor_copy(out=ag1r[:], in_=ag1[:])
                    a1s = pool.tile([P, K1 * P], dt.bfloat16, tag="a1s",
                                    name=f"a1s{s}")
                    for m in range(K1):
                        tp = psum1.tile([P, P], dt.float32, tag="tp1",
                                        name=f"tp1_{s}_{m}")
                        nc.tensor.transpose(tp[:], ag1r[:, m * P:(m + 1) * P],
                                            ident[:])
                        nc.vector.tensor_copy(out=a1s[:, m * P:(m + 1) * P],
                                              in_=tp[:])
                    zp = psum1.tile([P, HID_CH], dt.float32, tag="zp",
                                    name=f"zp{s}")
                    for m in range(K1):
                        nc.tensor.matmul(
                            out=zp[:],
                            lhsT=a1s[:, m * P:(m + 1) * P],
                            rhs=w1_t[:, m * HID_CH:(m + 1) * HID_CH],
                            start=(m == 0),
                            stop=(m == K1 - 1),
                        )
                    z_t = pool.tile([P, HID_CH], dt.bfloat16, tag="z",
                                    name=f"z{s}")
                    if has_bias:
                        nc.vector.tensor_add(out=zp[:], in0=zp[:], in1=b1_t[:])
                    nc.scalar.activation(out=z_t[:], in_=zp[:],
                                         func=mybir.ActivationFunctionType.Relu)
                    nc.sync.dma_start(out=z_slice[s * P:(s + 1) * P, :],
                                      in_=z_t[:])
                    if s == LO_SUPS - 1:
                        nc.gpsimd.collective_compute(
                            "AllGather", mybir.AluOpType.bypass,
                            replica_groups=[list(range(N_CORES))],
                            ins=[z_slice[0:LO, :]],
                            outs=[z_lo.opt()],
                        )
                nc.gpsimd.collective_compute(
                    "AllGather", mybir.AluOpType.bypass,
                    replica_groups=[list(range(N_CORES))],
                    ins=[z_slice[LO:ROWS_PER_CORE, :]],
                    outs=[z_hi.opt()],
                )

            # ---------------- Layer 2 pass A: lo-half srcs ----------------
            with tc.tile_pool(name="psumA", bufs=2, space="PSUM") as psumA:
                zidxlo_all = cpool.tile([P, SUP_PER_CORE * B2lo], dt.int32,
                                        name="zidxlo_all")
                nc.sync.dma_start(
                    out=zidxlo_all[:].rearrange("p (s b) -> p s b",
                                                s=SUP_PER_CORE),
                    in_=t_zidxlo[:].rearrange("s p b -> p s b"))
                for s in range(SUP_PER_CORE):
                    g_t = gpool.tile([P, B2lo * HID_CH], dt.bfloat16, tag="glo",
                                     name=f"glo{s}")
                    if s < 5:
                        nc.vector.memset(g_t[:], 0.0)
                    for b in range(B2lo):
                        nc.gpsimd.indirect_dma_start(
                            out=g_t[:, b * HID_CH:(b + 1) * HID_CH],
                            out_offset=None,
                            in_=z_lo[:],
                            in_offset=bass.IndirectOffsetOnAxis(
                                ap=zidxlo_all[:, s * B2lo + b:s * B2lo + b + 1],
                                axis=0),
                            bounds_check=N_CORES * LO - 1,
                            oob_is_err=False,
                        )
                    st_t = bigpool.tile([P, B2lo * P], dt.bfloat16, tag="st2lo",
                                        name=f"st2lo{s}")
                    nc.sync.dma_start(out=st_t[:], in_=t_st2lo[s])
                    agA = psumA.tile([P, HID_CH], dt.float32, tag="agA",
                                     name=f"agA_{s}")
                    for b in range(B2lo):
                        nc.tensor.matmul(
                            out=agA[:],
                            lhsT=st_t[:, b * P:(b + 1) * P],
                            rhs=g_t[:, b * HID_CH:(b + 1) * HID_CH],
                            start=(b == 0),
                            stop=(b == B2lo - 1),
                        )
                    pl_t = pool.tile([P, HID_CH], dt.bfloat16, tag="pl",
                                     name=f"pl{s}")
                    nc.vector.tensor_copy(out=pl_t[:], in_=agA[:])
                    nc.sync.dma_start(out=partial[s], in_=pl_t[:])

            # ---------------- Layer 2 pass B: hi-half + finish ----------------
            with tc.tile_pool(name="psumB", bufs=2, space="PSUM") as psumB:
                zidxhi_all = cpool.tile([P, SUP_PER_CORE * B2hi], dt.int32,
                                        name="zidxhi_all")
                nc.sync.dma_start(
                    out=zidxhi_all[:].rearrange("p (s b) -> p s b",
                                                s=SUP_PER_CORE),
                    in_=t_zidxhi[:].rearrange("s p b -> p s b"))
                for s in range(SUP_PER_CORE):
                    g_t = gpool.tile([P, B2hi * HID_CH], dt.bfloat16, tag="ghi",
                                     name=f"ghi{s}")
                    if s < 5:
                        nc.vector.memset(g_t[:], 0.0)
                    for b in range(B2hi):
                        nc.gpsimd.indirect_dma_start(
                            out=g_t[:, b * HID_CH:(b + 1) * HID_CH],
                            out_offset=None,
                            in_=z_hi[:],
                            in_offset=bass.IndirectOffsetOnAxis(
                                ap=zidxhi_all[:, s * B2hi + b:s * B2hi + b + 1],
                                axis=0),
                            bounds_check=N_CORES * HI - 1,
                            oob_is_err=False,
                        )
                    st_t = bigpool.tile([P, B2hi * P], dt.bfloat16, tag="st2hi",
                                        name=f"st2hi{s}")
                    nc.sync.dma_start(out=st_t[:], in_=t_st2hi[s])
                    agB = psumB.tile([P, HID_CH], dt.float32, tag="agB",
                                     name=f"agB_{s}")
                    for b in range(B2hi):
                        nc.tensor.matmul(
                            out=agB[:],
                            lhsT=st_t[:, b * P:(b + 1) * P],
                            rhs=g_t[:, b * HID_CH:(b + 1) * HID_CH],
                            start=(b == 0),
                            stop=(b == B2hi - 1),
                        )
                    # agg2 = agB + partial_lo + dinv2 * z_own
                    pl_t = pool.tile([P, HID_CH], dt.bfloat16, tag="plb",
                                     name=f"plb{s}")
                    nc.sync.dma_start(out=pl_t[:], in_=partial[s])
                    zown_t = pool.tile([P, HID_CH], dt.bfloat16, tag="zown",
                                       name=f"zown{s}")
                    nc.sync.dma_start(out=zown_t[:],
                                      in_=z_slice[s * P:(s + 1) * P, :])
                    dinv2_t = pool.tile([P, 1], dt.float32, tag="dinv2",
                                        name=f"dinv2{s}")
                    nc.sync.dma_start(out=dinv2_t[:], in_=t_dinv2[s])
                    ag2r = pool.tile([P, HID_CH], dt.float32, tag="ag2r",
                                     name=f"ag2r{s}")
                    nc.vector.scalar_tensor_tensor(
                        out=ag2r[:], in0=zown_t[:], scalar=dinv2_t[:, :1],
                        in1=agB[:], op0=mybir.AluOpType.mult,
                        op1=mybir.AluOpType.add,
                    )
                    nc.vector.tensor_add(out=ag2r[:], in0=ag2r[:], in1=pl_t[:])

                    a2s = pool.tile([P, K2 * P], dt.bfloat16, tag="a2s",
                                    name=f"a2s{s}")
                    for m in range(K2):
                        tp = psumB.tile([P, P], dt.float32, tag="tp2",
                                        name=f"tp2_{s}_{m}")
                        nc.tensor.transpose(tp[:], ag2r[:, m * P:(m + 1) * P],
                                            ident[:])
                        nc.vector.tensor_copy(out=a2s[:, m * P:(m + 1) * P],
                                              in_=tp[:])
                    op = psumB.tile([P, HID_CH], dt.float32, tag="op",
                                    name=f"op{s}")
                    for m in range(K2):
                        nc.tensor.matmul(
                            out=op[:],
                            lhsT=a2s[:, m * P:(m + 1) * P],
                            rhs=w2_t[:, m * HID_CH:(m + 1) * HID_CH],
                            start=(m == 0),
                            stop=(m == K2 - 1),
                        )
                    o_t = pool.tile([P, HID_CH], dt.float32, tag="o",
                                    name=f"o{s}")
                    if has_bias:
                        nc.vector.tensor_add(out=o_t[:], in0=op[:], in1=b2_t[:])
                    else:
                        nc.vector.tensor_copy(out=o_t[:], in_=op[:])
                    nc.sync.dma_start(out=t_out[s * P:(s + 1) * P, :], in_=o_t[:])

    nc.compile()
    return nc


def kernel(x, edge_index, W1, b1, W2, b2):
    global LAST_RESULTS
    from concourse import bass_utils

    x = np.asarray(x, np.float32)
    edge_index = np.asarray(edge_index)
    W1 = np.asarray(W1, np.float32)
    b1 = np.asarray(b1, np.float32)
    W2 = np.asarray(W2, np.float32)
    b2 = np.asarray(b2, np.float32)

    prep = _preprocess(x, edge_index)
    B1, B2lo, B2hi = prep["B1"], prep["B2lo"], prep["B2hi"]
    has_bias = bool(np.any(b1) or np.any(b2))

    key = (B1, B2lo, B2hi, has_bias)
    if key not in _COMPILED:
        _COMPILED[key] = _build_program(B1, B2lo, B2hi, has_bias)
    nc = _COMPILED[key]

    w1_in = np.ascontiguousarray(W1.astype(bf16).reshape(2, P, HID_CH))
    w2_in = np.ascontiguousarray(W2.astype(bf16).reshape(4, P, HID_CH))

    in_maps = []
    for c in range(N_CORES):
        s0, s1 = c * SUP_PER_CORE, (c + 1) * SUP_PER_CORE
        m = {
            "xe": np.ascontiguousarray(prep["xe"][s0:s1]),
            "st1": np.ascontiguousarray(prep["st1"][s0:s1]),
            "st2lo": np.ascontiguousarray(prep["st2lo"][s0:s1]),
            "st2hi": np.ascontiguousarray(prep["st2hi"][s0:s1]),
            "zidxlo": np.ascontiguousarray(prep["zidxlo"][s0:s1]),
            "zidxhi": np.ascontiguousarray(prep["zidxhi"][s0:s1]),
            "dinv2": np.ascontiguousarray(prep["dinv2"][s0:s1]),
            "w1": w1_in,
            "w2": w2_in,
        }
        if has_bias:
            m["b1b"] = np.tile(b1.astype(np.float32)[None, :], (P, 1))
            m["b2b"] = np.tile(b2.astype(np.float32)[None, :], (P, 1))
        in_maps.append(m)

    res = bass_utils.run_bass_kernel_spmd(
        nc, in_maps, core_ids=list(range(N_CORES)), trace=TRACE,
    )
    LAST_RESULTS = res

    out = np.concatenate([res.results[c]["out"] for c in range(N_CORES)], axis=0)
    return np.ascontiguousarray(out[:N_NODES]).astype(np.float32)
